# revision 1
# baseline (speedup 1.0000x reference)
"""DGCNN forward on 8 Trainium2 NeuronCores, data-parallel over batch.

Contract: kernel(**inputs) takes the FULL (unsharded) inputs from
reference.setup_inputs() and returns the FULL (32, 40) output.

Algorithm (exact, fp32):
  EdgeConv(x)_i = max_{j in knn20(i)} relu(bn(W @ [x_j - x_i; x_i]))
 decomposes (relu/max commute, bn is affine) into
  u_j = s*(wA @ x_j);  v_i = s*((wB-wA) @ x_i) + b
  out_i = relu( max_{j in knn20(i)} u_j  +  v_i )
 so each layer is: pairwise-distance matmul (PE) -> exact top-20 row
 selection (DVE max8/match_replace/max_index) -> gather u rows by index
 (GPSIMD ap_gather) -> windowed max (DVE reduce) -> +v, relu (ACT).
"""

import numpy as np

B, N, K = 32, 1024, 20
EPS = 1e-5
NCORES = 8
BPC = B // NCORES          # batches per core
NEG = -1e30

_CACHE = {}


# ---------------------------------------------------------------- weight prep
def _prep_weights(inp):
    """Fold BN into the edge-conv and MLP weights (numpy, host-side)."""
    w = {}
    couts = [64, 64, 64, 128]
    cins = [3, 64, 64, 64]
    for l in range(4):
        wl = inp[f'w{l+1}']            # (Cout, 2C)
        g = inp[f'g{l+1}']
        b = inp[f'b{l+1}']
        C = cins[l]
        s = g / np.sqrt(1.0 + EPS)
        wA = wl[:, :C]                  # acts on (x_j - x_i)
        wB = wl[:, C:]                  # acts on x_i
        Wu = (s[:, None] * wA).T.astype(np.float32)           # (C, Cout)
        Wv = (s[:, None] * (wB - wA)).T.astype(np.float32)    # (C, Cout)
        cout = couts[l]
        if l < 3:
            # batch-pair packing: [Wu | 0] and [0 | Wu], (C, 128)
            zu = np.zeros((C, 64), np.float32)
            w[f'wu{l}a'] = np.concatenate([Wu, zu], 1)
            w[f'wu{l}b'] = np.concatenate([zu, Wu], 1)
            w[f'wv{l}a'] = np.concatenate([Wv, zu], 1)
            w[f'wv{l}b'] = np.concatenate([zu, Wv], 1)
            w[f'bv{l}'] = np.concatenate([b, b]).reshape(128, 1).astype(np.float32)
        else:
            w[f'wu{l}'] = Wu            # (64, 128)
            w[f'wv{l}'] = Wv
            w[f'bv{l}'] = b.reshape(128, 1).astype(np.float32)
    s5 = inp['g5'] / np.sqrt(1.0 + EPS)
    w['w1t'] = (s5[:, None] * inp['lw1']).T.astype(np.float32)      # (320, 1024)
    w['b1'] = (s5 * inp['lb1'] + inp['b5']).reshape(8, 128).T.astype(np.float32).copy()  # (128, 8)
    s6 = inp['g6'] / np.sqrt(1.0 + EPS)
    w['w2t'] = (s6[:, None] * inp['lw2']).T.astype(np.float32)      # (1024, 512)
    w['b2'] = (s6 * inp['lb2'] + inp['b6']).reshape(4, 128).T.astype(np.float32).copy()  # (128, 4)
    w['w3t'] = inp['lw3'].T.astype(np.float32)                      # (512, 40)
    w['b3'] = inp['lb3'].reshape(40, 1).astype(np.float32)
    return w


# ---------------------------------------------------------------- bass program
def _build_program(n_layers=4, with_mlp=True):
    import concourse.bass as bass
    import concourse.bacc as bacc
    import concourse.mybir as mybir
    from concourse.tile import TileContext

    f32 = mybir.dt.float32
    u16 = mybir.dt.uint16
    i16 = mybir.dt.int16
    AF = mybir.ActivationFunctionType
    AX = mybir.AxisListType

    nc = bacc.Bacc("TRN2")

    # ---- DRAM tensors (per-core inputs) ----
    xT = nc.dram_tensor("xT", [BPC, 3, N], f32, kind="ExternalInput").ap()
    cins = [3, 64, 64, 64]
    couts = [64, 64, 64, 128]
    wt = {}
    for l in range(3):
        for nm in ('wua', 'wub', 'wva', 'wvb'):
            key = f'{nm[:2]}{l}{nm[2]}'
            wt[key] = nc.dram_tensor(key, [cins[l], 128], f32, kind="ExternalInput").ap()
        wt[f'bv{l}'] = nc.dram_tensor(f'bv{l}', [128, 1], f32, kind="ExternalInput").ap()
    wt['wu3'] = nc.dram_tensor('wu3', [64, 128], f32, kind="ExternalInput").ap()
    wt['wv3'] = nc.dram_tensor('wv3', [64, 128], f32, kind="ExternalInput").ap()
    wt['bv3'] = nc.dram_tensor('bv3', [128, 1], f32, kind="ExternalInput").ap()
    w1t = nc.dram_tensor("w1t", [320, 1024], f32, kind="ExternalInput").ap()
    b1 = nc.dram_tensor("b1", [128, 8], f32, kind="ExternalInput").ap()
    w2t = nc.dram_tensor("w2t", [1024, 512], f32, kind="ExternalInput").ap()
    b2 = nc.dram_tensor("b2", [128, 4], f32, kind="ExternalInput").ap()
    w3t = nc.dram_tensor("w3t", [512, 40], f32, kind="ExternalInput").ap()
    b3 = nc.dram_tensor("b3", [40, 1], f32, kind="ExternalInput").ap()

    out_d = nc.dram_tensor("out", [40, BPC], f32, kind="ExternalOutput").ap()
    stage = nc.dram_tensor("idx_stage", [BPC, N, K], u16, kind="Internal").ap()
    pooled_d = nc.dram_tensor("pooled_stage", [BPC, 320], f32, kind="Internal").ap()

    NPAIR = BPC // 2

    with TileContext(nc) as tc:
        with (
            tc.tile_pool(name="const", bufs=1) as cpool,
            tc.tile_pool(name="wpool", bufs=1) as wpool,
            tc.tile_pool(name="feat", bufs=1) as fpool,
            tc.tile_pool(name="work", bufs=2) as wkpool,
            tc.tile_pool(name="pdp", bufs=6) as pdpool,
            tc.tile_pool(name="sel", bufs=6) as selpool,
            tc.tile_pool(name="gath", bufs=2) as gpool,
            tc.tile_pool(name="ps", bufs=2, space="PSUM") as pspool,
            tc.tile_pool(name="psx", bufs=1, space="PSUM") as psxpool,
        ):
            ones_col = cpool.tile([128, 1], f32, tag="onesc")
            nc.vector.memset(ones_col[:, :], 1.0)
            ones_row = cpool.tile([1, N], f32, tag="onesr")
            nc.vector.memset(ones_row[:, :], 1.0)

            # load weights (all at base partition 0 — the PE requires matmul
            # operands to share a base partition, and mixing tile_positions
            # inside one PSUM accumulation group faults on HW)
            wsb = {}
            for l in range(3):
                for key in (f'wu{l}a', f'wu{l}b', f'wv{l}a', f'wv{l}b'):
                    t = wpool.tile([cins[l], 128], f32, tag=key, name=key)
                    nc.sync.dma_start(t[:, :], wt[key])
                    wsb[key] = t
                t = wpool.tile([128, 1], f32, tag=f'bv{l}', name=f'bv{l}')
                nc.sync.dma_start(t[:, :], wt[f'bv{l}'])
                wsb[f'bv{l}'] = t
            for key in ('wu3', 'wv3'):
                t = wpool.tile([64, 128], f32, tag=key, name=key)
                nc.sync.dma_start(t[:, :], wt[key])
                wsb[key] = t
            t = wpool.tile([128, 1], f32, tag='bv3', name='bv3')
            nc.sync.dma_start(t[:, :], wt['bv3'])
            wsb['bv3'] = t

            # Feature state per pair: paired tile F[p] (128, N) holds unit A
            # in partitions [0:64); FB[p] (64, N) is unit B's copy at base 0
            # (extracted by DMA) so every matmul operand starts at partition 0.
            F = [fpool.tile([128, N], f32, tag=f"F{p}", name=f"F{p}", bufs=2)
                 for p in range(NPAIR)]
            FB = [fpool.tile([64, N], f32, tag=f"FB{p}", name=f"FB{p}", bufs=2)
                  for p in range(NPAIR)]
            for p in range(NPAIR):
                nc.sync.dma_start(F[p][0:3, :], xT[2 * p, :, :])
                nc.sync.dma_start(FB[p][0:3, :], xT[2 * p + 1, :, :])

            for l in range(n_layers):
                C = cins[l]
                for p in range(NPAIR):
                    Fp = F[p]
                    FBp = FB[p]
                    funits = (Fp, FBp)  # unit -> feature AP source (base 0)
                    # ---- squared norms (per unit, base partition 0) ----
                    negxx = [None, None]
                    for ui in range(2):
                        fsq = wkpool.tile([64, N], f32, tag=f"fsq{ui}",
                                          name=f"fsq{ui}")
                        nc.scalar.activation(fsq[0:C, :], funits[ui][0:C, :], AF.Square)
                        xxp = psxpool.tile([1, N], f32, tag="xx", name="xxp")
                        for h in range(2):
                            sl = slice(h * 512, (h + 1) * 512)
                            nc.tensor.matmul(xxp[:, sl], ones_col[0:C, :],
                                             fsq[0:C, sl], start=True, stop=True)
                        nxx = wkpool.tile([1, N], f32, tag=f"nxx{ui}", name=f"nxx{ui}")
                        nc.scalar.activation(nxx[:, :], xxp[:, :], AF.Copy, scale=-1.0)
                        negxx[ui] = nxx

                    # ---- u/v feature tables ----
                    if l < 3:
                        # batch-pair packed: psum = [u_A ; u_B] via padded weights
                        upair = wkpool.tile([128, N], f32, tag="upair")
                        vpair = wkpool.tile([128, N], f32, tag="vpair")
                        for h in range(2):
                            sl = slice(h * 512, (h + 1) * 512)
                            up = pspool.tile([128, 512], f32, tag="acc")
                            vp = pspool.tile([128, 512], f32, tag="acc")
                            nc.tensor.matmul(up[:, :], wsb[f'wu{l}a'][:, :], Fp[0:C, sl],
                                             start=True, stop=False)
                            nc.tensor.matmul(up[:, :], wsb[f'wu{l}b'][:, :],
                                             FBp[0:C, sl], start=False, stop=True)
                            nc.tensor.matmul(vp[:, :], wsb[f'wv{l}a'][:, :], Fp[0:C, sl],
                                             start=True, stop=False)
                            nc.tensor.matmul(vp[:, :], wsb[f'wv{l}b'][:, :],
                                             FBp[0:C, sl], start=False, stop=True)
                            nc.scalar.activation(upair[:, sl], up[:, :], AF.Copy)
                            nc.scalar.activation(vpair[:, sl], vp[:, :], AF.Identity,
                                                 bias=wsb[f'bv{l}'][:, :])
                            del up, vp
                    else:
                        # layer 4: Cout=128 -> per-unit full-width tables
                        u4s, v4s = [], []
                        for ui in range(2):
                            u4 = wkpool.tile([128, N], f32, tag="upair", name=f"u4_{ui}")
                            v4 = wkpool.tile([128, N], f32, tag="vpair", name=f"v4_{ui}")
                            for h in range(2):
                                sl = slice(h * 512, (h + 1) * 512)
                                up = pspool.tile([128, 512], f32, tag="acc")
                                vp = pspool.tile([128, 512], f32, tag="acc")
                                nc.tensor.matmul(up[:, :], wsb['wu3'][:, :],
                                                 funits[ui][0:C, sl], start=True, stop=True)
                                nc.tensor.matmul(vp[:, :], wsb['wv3'][:, :],
                                                 funits[ui][0:C, sl], start=True, stop=True)
                                nc.scalar.activation(u4[:, sl], up[:, :], AF.Copy)
                                nc.scalar.activation(v4[:, sl], vp[:, :], AF.Identity,
                                                     bias=wsb['bv3'][:, :])
                                del up, vp
                            u4s.append(u4)
                            v4s.append(v4)

                    for ui in range(2):
                        b = 2 * p + ui
                        FX = funits[ui]
                        # ---- pd + top-20 selection per 128-row chunk ----
                        for ic in range(8):
                            isl = slice(ic * 128, (ic + 1) * 128)
                            pdp = pspool.tile([128, 1024], f32, tag="pd")
                            for h in range(2):
                                sl = slice(h * 512, (h + 1) * 512)
                                nc.tensor.matmul(pdp[:, sl], FX[0:C, isl],
                                                 FX[0:C, sl], start=True, stop=False)
                                nc.tensor.matmul(pdp[:, sl], FX[0:C, isl],
                                                 FX[0:C, sl], start=False, stop=False)
                                nc.tensor.matmul(pdp[:, sl], negxx[ui][:, isl],
                                                 ones_row[:, sl], start=False, stop=False)
                                nc.tensor.matmul(pdp[:, sl], ones_row[:, isl],
                                                 negxx[ui][:, sl], start=False, stop=True)
                            pda = pdpool.tile([128, 1024], f32, tag="pda")
                            nc.scalar.activation(pda[:, :], pdp[:, :], AF.Copy)
                            del pdp

                            v0 = selpool.tile([128, 8], f32, tag="v0")
                            v1 = selpool.tile([128, 8], f32, tag="v1")
                            v2 = selpool.tile([128, 8], f32, tag="v2")
                            i0 = selpool.tile([128, 8], u16, tag="i0")
                            i1 = selpool.tile([128, 8], u16, tag="i1")
                            i2 = selpool.tile([128, 8], u16, tag="i2")
                            nc.vector.max(out=v0[:, :], in_=pda[:, :])
                            nc.vector.max_index(out=i0[:, :], in_max=v0[:, :], in_values=pda[:, :])
                            pdb = pdpool.tile([128, 1024], f32, tag="pdb")
                            nc.vector.match_replace(out=pdb[:, :], in_to_replace=v0[:, :],
                                                    in_values=pda[:, :], imm_value=NEG)
                            nc.vector.max(out=v1[:, :], in_=pdb[:, :])
                            nc.vector.max_index(out=i1[:, :], in_max=v1[:, :], in_values=pdb[:, :])
                            nc.vector.match_replace(out=pda[:, :], in_to_replace=v1[:, :],
                                                    in_values=pdb[:, :], imm_value=NEG)
                            nc.vector.max(out=v2[:, :], in_=pda[:, :])
                            nc.vector.max_index(out=i2[:, :], in_max=v2[:, :], in_values=pda[:, :])
                            # stage the 20 indices with 3 DMAs (SP engine) so
                            # the DVE does no assembly copies
                            nc.sync.dma_start(stage[b, isl, 0:8], i0[:, :])
                            nc.sync.dma_start(stage[b, isl, 8:16], i1[:, :])
                            nc.sync.dma_start(stage[b, isl, 16:20], i2[:, 0:4])

                    # ---- gather + window-max + v + relu ----
                    def window_max(G, out_slice):
                        nc.vector.reduce_max(
                            out=out_slice,
                            in_=G.rearrange("p (i t) -> p i t", t=K),
                            axis=AX.X)

                    if l < 3:
                        wrap = gpool.tile([128, N * K // 16], u16, tag="wrap")
                        for g in range(8):
                            bsrc = 2 * p + (0 if g < 4 else 1)
                            lin = stage[bsrc].rearrange("i t -> (i t)").rearrange(
                                "(c r) -> r c", r=16)
                            nc.sync.dma_start(wrap[g * 16:(g + 1) * 16, :], lin)
                        Mp = wkpool.tile([128, N], f32, tag="Mp")
                        for gc in range(8):
                            G = gpool.tile([128, 2560], f32, tag="G", bufs=3)
                            nc.gpsimd.ap_gather(
                                out_ap=G[:, :], in_ap=upair[:, :],
                                idxs_ap=wrap[:, gc * 160:(gc + 1) * 160].bitcast(i16),
                                channels=128, num_elems=N, d=1, num_idxs=2560)
                            window_max(G, Mp[:, gc * 128:(gc + 1) * 128])
                        nc.vector.tensor_add(Mp[:, :], Mp[:, :], vpair[:, :])
                        Fnext = fpool.tile([128, N], f32, tag=f"F{p}",
                                           name=f"F{p}_{l}", bufs=2)
                        nc.scalar.activation(Fnext[:, :], Mp[:, :], AF.Relu)
                        FBnext = fpool.tile([64, N], f32, tag=f"FB{p}",
                                            name=f"FB{p}_{l}", bufs=2)
                        nc.sync.dma_start(FBnext[:, :], Fnext[64:128, :])
                        # global max-pool for this layer
                        gp = selpool.tile([128, 1], f32, tag="gp")
                        nc.vector.reduce_max(out=gp[:, :], in_=Fnext[:, :], axis=AX.X)
                        nc.sync.dma_start(pooled_d[2 * p, l * 64:(l + 1) * 64], gp[0:64, :])
                        nc.sync.dma_start(pooled_d[2 * p + 1, l * 64:(l + 1) * 64], gp[64:128, :])
                        F[p], FB[p] = Fnext, FBnext
                    else:
                        for ui in range(2):
                            b = 2 * p + ui
                            wrap = gpool.tile([128, N * K // 16], u16, tag="wrap")
                            lin = stage[b].rearrange("i t -> (i t)").rearrange(
                                "(c r) -> r c", r=16)
                            for g in range(8):
                                nc.sync.dma_start(wrap[g * 16:(g + 1) * 16, :], lin)
                            Mp = wkpool.tile([128, N], f32, tag="Mp")
                            for gc in range(8):
                                G = gpool.tile([128, 2560], f32, tag="G", bufs=3)
                                nc.gpsimd.ap_gather(
                                    out_ap=G[:, :], in_ap=u4s[ui][:, :],
                                    idxs_ap=wrap[:, gc * 160:(gc + 1) * 160].bitcast(i16),
                                    channels=128, num_elems=N, d=1, num_idxs=2560)
                                window_max(G, Mp[:, gc * 128:(gc + 1) * 128])
                            nc.vector.tensor_add(Mp[:, :], Mp[:, :], v4s[ui][:, :])
                            x4t = wkpool.tile([128, N], f32, tag="x4t")
                            nc.scalar.activation(x4t[:, :], Mp[:, :], AF.Relu)
                            gp = selpool.tile([128, 1], f32, tag="gp")
                            nc.vector.reduce_max(out=gp[:, :], in_=x4t[:, :], axis=AX.X)
                            nc.sync.dma_start(pooled_d[b, 192:320], gp[:, :])

        # ================= MLP head (own pool scope) =================
        if not with_mlp:
            with tc.tile_pool(name="stub", bufs=1) as spool:
                so = spool.tile([40, BPC], f32, name="so")
                nc.sync.dma_start(so[:, :], pooled_d[:, 0:40].rearrange("b p -> p b"))
                nc.sync.dma_start(out_d, so[:, :])
        elif True:
          with (
            tc.tile_pool(name="mlp", bufs=1) as mpool,
            tc.tile_pool(name="mps", bufs=2, space="PSUM") as mpspool,
          ):
            pooledT = mpool.tile([128, 3, BPC], f32, tag="pooledT")
            for kc in range(3):
                kn = 128 if kc < 2 else 64
                nc.sync.dma_start(pooledT[0:kn, kc, :],
                                  pooled_d[:, kc * 128:kc * 128 + kn].rearrange("b p -> p b"))
            w1sb = mpool.tile([128, 3, 1024], f32, tag="w1sb")
            for kc in range(3):
                kn = 128 if kc < 2 else 64
                nc.sync.dma_start(w1sb[0:kn, kc, :], w1t[kc * 128:kc * 128 + kn, :])
            b1sb = mpool.tile([128, 8], f32, tag="b1sb")
            nc.sync.dma_start(b1sb[:, :], b1)
            h1 = mpool.tile([128, 8, BPC], f32, tag="h1")
            for mc in range(8):
                hp = mpspool.tile([128, BPC], f32, tag="acc")
                for kc in range(3):
                    kn = 128 if kc < 2 else 64
                    nc.tensor.matmul(hp[:, :], w1sb[0:kn, kc, mc * 128:(mc + 1) * 128],
                                     pooledT[0:kn, kc, :], start=(kc == 0), stop=(kc == 2))
                nc.scalar.activation(h1[:, mc, :], hp[:, :], AF.Relu,
                                     bias=b1sb[:, mc:mc + 1])
            w2sb = mpool.tile([128, 8, 512], f32, tag="w2sb")
            for kc in range(8):
                nc.sync.dma_start(w2sb[:, kc, :], w2t[kc * 128:(kc + 1) * 128, :])
            b2sb = mpool.tile([128, 4], f32, tag="b2sb")
            nc.sync.dma_start(b2sb[:, :], b2)
            h2 = mpool.tile([128, 4, BPC], f32, tag="h2")
            for mc in range(4):
                hp = mpspool.tile([128, BPC], f32, tag="acc")
                for kc in range(8):
                    nc.tensor.matmul(hp[:, :], w2sb[:, kc, mc * 128:(mc + 1) * 128],
                                     h1[:, kc, :], start=(kc == 0), stop=(kc == 7))
                nc.scalar.activation(h2[:, mc, :], hp[:, :], AF.Relu,
                                     bias=b2sb[:, mc:mc + 1])
            w3sb = mpool.tile([128, 4, 40], f32, tag="w3sb")
            for kc in range(4):
                nc.sync.dma_start(w3sb[:, kc, :], w3t[kc * 128:(kc + 1) * 128, :])
            b3sb = mpool.tile([40, 1], f32, tag="b3sb")
            nc.sync.dma_start(b3sb[:, :], b3)
            outp = mpspool.tile([40, BPC], f32, tag="acc")
            for kc in range(4):
                nc.tensor.matmul(outp[:, :], w3sb[:, kc, :], h2[:, kc, :],
                                 start=(kc == 0), stop=(kc == 3))
            outsb = mpool.tile([40, BPC], f32, tag="outsb")
            nc.scalar.activation(outsb[:, :], outp[:, :], AF.Identity, bias=b3sb[:, :])
            nc.sync.dma_start(out_d, outsb[:, :])

    nc.compile()
    return nc


# ---------------------------------------------------------------- entry point
def _run(inputs, **spmd_kwargs):
    key = "prog"
    if key not in _CACHE:
        _CACHE[key] = _build_program()
    nc = _CACHE[key]

    inputs = {k: np.asarray(v) for k, v in inputs.items()}
    w = _prep_weights(inputs)
    x = np.asarray(inputs['x'], dtype=np.float32)   # (32, 1024, 3)
    in_maps = []
    for c in range(NCORES):
        xs = x[c * BPC:(c + 1) * BPC]                       # (4, 1024, 3)
        m = {'xT': np.ascontiguousarray(xs.transpose(0, 2, 1)).astype(np.float32)}
        m.update({k: np.ascontiguousarray(v) for k, v in w.items()})
        in_maps.append(m)

    from concourse.bass_utils import run_bass_kernel_spmd
    res = run_bass_kernel_spmd(nc, in_maps, core_ids=list(range(NCORES)), **spmd_kwargs)
    out = np.concatenate([r['out'].T for r in res.results], axis=0)  # (32, 40)
    return out.astype(np.float32), res


def kernel(**inputs):
    return _run(inputs)[0]



# revision 2
# speedup vs baseline: 17.2827x; 17.2827x over previous
"""DGCNN forward on 8 Trainium2 NeuronCores, data-parallel over batch.

Contract: kernel(**inputs) takes the FULL (unsharded) inputs from
reference.setup_inputs() and returns the FULL (32, 40) output.

Algorithm (exact, fp32):
  EdgeConv(x)_i = max_{j in knn20(i)} relu(bn(W @ [x_j - x_i; x_i]))
 decomposes (relu/max commute, bn is affine) into
  u_j = s*(wA @ x_j);  v_i = s*((wB-wA) @ x_i) + b
  out_i = relu( max_{j in knn20(i)} u_j  +  v_i )
 so each layer is: pairwise-distance matmul (PE) -> exact top-20 row
 selection (DVE max8/match_replace/max_index) -> gather u rows by index
 (GPSIMD ap_gather) -> windowed max (DVE reduce) -> +v, relu (ACT).

Host path: the compiled SPMD executable, the sharding mesh, and the
device-resident weight buffers are all cached across calls; a call only
ships the point cloud x (48 KiB/core), launches, and fetches the (40,
BPC) logits per core.  Weight inputs are verified against the cached
copy (exact bytewise compare) and re-uploaded if they changed.
"""

import numpy as np

B, N, K = 32, 1024, 20
EPS = 1e-5
NCORES = 8
BPC = B // NCORES          # batches per core
NEG = -1e30

_CACHE = {}

_WNAMES = ['w1', 'g1', 'b1', 'w2', 'g2', 'b2', 'w3', 'g3', 'b3',
           'w4', 'g4', 'b4', 'lw1', 'lb1', 'g5', 'b5', 'lw2', 'lb2',
           'g6', 'b6', 'lw3', 'lb3']


# ---------------------------------------------------------------- weight prep
def _prep_weights(inp):
    """Fold BN into the edge-conv and MLP weights (numpy, host-side)."""
    w = {}
    couts = [64, 64, 64, 128]
    cins = [3, 64, 64, 64]
    for l in range(4):
        wl = inp[f'w{l+1}']            # (Cout, 2C)
        g = inp[f'g{l+1}']
        b = inp[f'b{l+1}']
        C = cins[l]
        s = g / np.sqrt(1.0 + EPS)
        wA = wl[:, :C]                  # acts on (x_j - x_i)
        wB = wl[:, C:]                  # acts on x_i
        Wu = (s[:, None] * wA).T.astype(np.float32)           # (C, Cout)
        Wv = (s[:, None] * (wB - wA)).T.astype(np.float32)    # (C, Cout)
        cout = couts[l]
        if l < 3:
            # batch-pair packing: [Wu | 0] and [0 | Wu], (C, 128)
            zu = np.zeros((C, 64), np.float32)
            w[f'wu{l}a'] = np.concatenate([Wu, zu], 1)
            w[f'wu{l}b'] = np.concatenate([zu, Wu], 1)
            w[f'wv{l}a'] = np.concatenate([Wv, zu], 1)
            w[f'wv{l}b'] = np.concatenate([zu, Wv], 1)
            w[f'bv{l}'] = np.concatenate([b, b]).reshape(128, 1).astype(np.float32)
        else:
            w[f'wu{l}'] = Wu            # (64, 128)
            w[f'wv{l}'] = Wv
            w[f'bv{l}'] = b.reshape(128, 1).astype(np.float32)
    s5 = inp['g5'] / np.sqrt(1.0 + EPS)
    w['w1t'] = (s5[:, None] * inp['lw1']).T.astype(np.float32)      # (320, 1024)
    w['b1'] = (s5 * inp['lb1'] + inp['b5']).reshape(8, 128).T.astype(np.float32).copy()  # (128, 8)
    s6 = inp['g6'] / np.sqrt(1.0 + EPS)
    w['w2t'] = (s6[:, None] * inp['lw2']).T.astype(np.float32)      # (1024, 512)
    w['b2'] = (s6 * inp['lb2'] + inp['b6']).reshape(4, 128).T.astype(np.float32).copy()  # (128, 4)
    w['w3t'] = inp['lw3'].T.astype(np.float32)                      # (512, 40)
    w['b3'] = inp['lb3'].reshape(40, 1).astype(np.float32)
    return w


# ---------------------------------------------------------------- bass program
def _build_program(n_layers=4, with_mlp=True, with_gather=True):
    import concourse.bass as bass
    import concourse.bacc as bacc
    import concourse.mybir as mybir
    from concourse.tile import TileContext

    f32 = mybir.dt.float32
    u16 = mybir.dt.uint16
    i16 = mybir.dt.int16
    AF = mybir.ActivationFunctionType
    AX = mybir.AxisListType

    nc = bacc.Bacc("TRN2")

    # ---- DRAM tensors (per-core inputs) ----
    xT = nc.dram_tensor("xT", [BPC, 3, N], f32, kind="ExternalInput").ap()
    cins = [3, 64, 64, 64]
    couts = [64, 64, 64, 128]
    wt = {}
    for l in range(3):
        for nm in ('wua', 'wub', 'wva', 'wvb'):
            key = f'{nm[:2]}{l}{nm[2]}'
            wt[key] = nc.dram_tensor(key, [cins[l], 128], f32, kind="ExternalInput").ap()
        wt[f'bv{l}'] = nc.dram_tensor(f'bv{l}', [128, 1], f32, kind="ExternalInput").ap()
    wt['wu3'] = nc.dram_tensor('wu3', [64, 128], f32, kind="ExternalInput").ap()
    wt['wv3'] = nc.dram_tensor('wv3', [64, 128], f32, kind="ExternalInput").ap()
    wt['bv3'] = nc.dram_tensor('bv3', [128, 1], f32, kind="ExternalInput").ap()
    w1t = nc.dram_tensor("w1t", [320, 1024], f32, kind="ExternalInput").ap()
    b1 = nc.dram_tensor("b1", [128, 8], f32, kind="ExternalInput").ap()
    w2t = nc.dram_tensor("w2t", [1024, 512], f32, kind="ExternalInput").ap()
    b2 = nc.dram_tensor("b2", [128, 4], f32, kind="ExternalInput").ap()
    w3t = nc.dram_tensor("w3t", [512, 40], f32, kind="ExternalInput").ap()
    b3 = nc.dram_tensor("b3", [40, 1], f32, kind="ExternalInput").ap()

    out_d = nc.dram_tensor("out", [40, BPC], f32, kind="ExternalOutput").ap()
    stage = nc.dram_tensor("idx_stage", [BPC, N, K], u16, kind="Internal").ap()
    pooled_d = nc.dram_tensor("pooled_stage", [BPC, 320], f32, kind="Internal").ap()

    NPAIR = BPC // 2

    with TileContext(nc) as tc:
        with (
            tc.tile_pool(name="const", bufs=1) as cpool,
            tc.tile_pool(name="wpool", bufs=1) as wpool,
            tc.tile_pool(name="feat", bufs=1) as fpool,
            tc.tile_pool(name="work", bufs=2) as wkpool,
            tc.tile_pool(name="pdp", bufs=6) as pdpool,
            tc.tile_pool(name="sel", bufs=6) as selpool,
            tc.tile_pool(name="gath", bufs=2) as gpool,
            tc.tile_pool(name="ps", bufs=2, space="PSUM") as pspool,
            tc.tile_pool(name="psx", bufs=1, space="PSUM") as psxpool,
        ):
            ones_col = cpool.tile([128, 1], f32, tag="onesc")
            nc.vector.memset(ones_col[:, :], 1.0)
            ones_row = cpool.tile([1, N], f32, tag="onesr")
            nc.vector.memset(ones_row[:, :], 1.0)

            # load weights (all at base partition 0 — the PE requires matmul
            # operands to share a base partition, and mixing tile_positions
            # inside one PSUM accumulation group faults on HW)
            wsb = {}
            for l in range(3):
                for key in (f'wu{l}a', f'wu{l}b', f'wv{l}a', f'wv{l}b'):
                    t = wpool.tile([cins[l], 128], f32, tag=key, name=key)
                    nc.sync.dma_start(t[:, :], wt[key])
                    wsb[key] = t
                t = wpool.tile([128, 1], f32, tag=f'bv{l}', name=f'bv{l}')
                nc.sync.dma_start(t[:, :], wt[f'bv{l}'])
                wsb[f'bv{l}'] = t
            for key in ('wu3', 'wv3'):
                t = wpool.tile([64, 128], f32, tag=key, name=key)
                nc.sync.dma_start(t[:, :], wt[key])
                wsb[key] = t
            t = wpool.tile([128, 1], f32, tag='bv3', name='bv3')
            nc.sync.dma_start(t[:, :], wt['bv3'])
            wsb['bv3'] = t

            # Feature state per pair: paired tile F[p] (128, N) holds unit A
            # in partitions [0:64); FB[p] (64, N) is unit B's copy at base 0
            # (extracted by DMA) so every matmul operand starts at partition 0.
            F = [fpool.tile([128, N], f32, tag=f"F{p}", name=f"F{p}", bufs=2)
                 for p in range(NPAIR)]
            FB = [fpool.tile([64, N], f32, tag=f"FB{p}", name=f"FB{p}", bufs=2)
                  for p in range(NPAIR)]
            for p in range(NPAIR):
                nc.sync.dma_start(F[p][0:3, :], xT[2 * p, :, :])
                nc.sync.dma_start(FB[p][0:3, :], xT[2 * p + 1, :, :])

            for l in range(n_layers):
                C = cins[l]
                for p in range(NPAIR):
                    Fp = F[p]
                    FBp = FB[p]
                    funits = (Fp, FBp)  # unit -> feature AP source (base 0)
                    # ---- squared norms (per unit, base partition 0) ----
                    negxx = [None, None]
                    for ui in range(2):
                        fsq = wkpool.tile([64, N], f32, tag=f"fsq{ui}",
                                          name=f"fsq{ui}")
                        nc.scalar.activation(fsq[0:C, :], funits[ui][0:C, :], AF.Square)
                        xxp = psxpool.tile([1, N], f32, tag="xx", name="xxp")
                        for h in range(2):
                            sl = slice(h * 512, (h + 1) * 512)
                            nc.tensor.matmul(xxp[:, sl], ones_col[0:C, :],
                                             fsq[0:C, sl], start=True, stop=True)
                        nxx = wkpool.tile([1, N], f32, tag=f"nxx{ui}", name=f"nxx{ui}")
                        nc.scalar.activation(nxx[:, :], xxp[:, :], AF.Copy, scale=-1.0)
                        negxx[ui] = nxx

                    # ---- u/v feature tables ----
                    if l < 3:
                        # batch-pair packed: psum = [u_A ; u_B] via padded weights
                        upair = wkpool.tile([128, N], f32, tag="upair")
                        vpair = wkpool.tile([128, N], f32, tag="vpair")
                        for h in range(2):
                            sl = slice(h * 512, (h + 1) * 512)
                            up = pspool.tile([128, 512], f32, tag="acc")
                            vp = pspool.tile([128, 512], f32, tag="acc")
                            nc.tensor.matmul(up[:, :], wsb[f'wu{l}a'][:, :], Fp[0:C, sl],
                                             start=True, stop=False)
                            nc.tensor.matmul(up[:, :], wsb[f'wu{l}b'][:, :],
                                             FBp[0:C, sl], start=False, stop=True)
                            nc.tensor.matmul(vp[:, :], wsb[f'wv{l}a'][:, :], Fp[0:C, sl],
                                             start=True, stop=False)
                            nc.tensor.matmul(vp[:, :], wsb[f'wv{l}b'][:, :],
                                             FBp[0:C, sl], start=False, stop=True)
                            nc.scalar.activation(upair[:, sl], up[:, :], AF.Copy)
                            nc.scalar.activation(vpair[:, sl], vp[:, :], AF.Identity,
                                                 bias=wsb[f'bv{l}'][:, :])
                            del up, vp
                    else:
                        # layer 4: Cout=128 -> per-unit full-width tables
                        u4s, v4s = [], []
                        for ui in range(2):
                            u4 = wkpool.tile([128, N], f32, tag="upair", name=f"u4_{ui}")
                            v4 = wkpool.tile([128, N], f32, tag="vpair", name=f"v4_{ui}")
                            for h in range(2):
                                sl = slice(h * 512, (h + 1) * 512)
                                up = pspool.tile([128, 512], f32, tag="acc")
                                vp = pspool.tile([128, 512], f32, tag="acc")
                                nc.tensor.matmul(up[:, :], wsb['wu3'][:, :],
                                                 funits[ui][0:C, sl], start=True, stop=True)
                                nc.tensor.matmul(vp[:, :], wsb['wv3'][:, :],
                                                 funits[ui][0:C, sl], start=True, stop=True)
                                nc.scalar.activation(u4[:, sl], up[:, :], AF.Copy)
                                nc.scalar.activation(v4[:, sl], vp[:, :], AF.Identity,
                                                     bias=wsb['bv3'][:, :])
                                del up, vp
                            u4s.append(u4)
                            v4s.append(v4)

                    for ui in range(2):
                        b = 2 * p + ui
                        FX = funits[ui]
                        # ---- pd + top-20 selection per 128-row chunk ----
                        for ic in range(8):
                            isl = slice(ic * 128, (ic + 1) * 128)
                            pdp = pspool.tile([128, 1024], f32, tag="pd")
                            for h in range(2):
                                sl = slice(h * 512, (h + 1) * 512)
                                nc.tensor.matmul(pdp[:, sl], FX[0:C, isl],
                                                 FX[0:C, sl], start=True, stop=False)
                                nc.tensor.matmul(pdp[:, sl], FX[0:C, isl],
                                                 FX[0:C, sl], start=False, stop=False)
                                nc.tensor.matmul(pdp[:, sl], negxx[ui][:, isl],
                                                 ones_row[:, sl], start=False, stop=False)
                                nc.tensor.matmul(pdp[:, sl], ones_row[:, isl],
                                                 negxx[ui][:, sl], start=False, stop=True)
                            pda = pdpool.tile([128, 1024], f32, tag="pda")
                            nc.scalar.activation(pda[:, :], pdp[:, :], AF.Copy)
                            del pdp

                            v0 = selpool.tile([128, 8], f32, tag="v0")
                            v1 = selpool.tile([128, 8], f32, tag="v1")
                            v2 = selpool.tile([128, 8], f32, tag="v2")
                            i0 = selpool.tile([128, 8], u16, tag="i0")
                            i1 = selpool.tile([128, 8], u16, tag="i1")
                            i2 = selpool.tile([128, 8], u16, tag="i2")
                            nc.vector.max(out=v0[:, :], in_=pda[:, :])
                            nc.vector.max_index(out=i0[:, :], in_max=v0[:, :], in_values=pda[:, :])
                            pdb = pdpool.tile([128, 1024], f32, tag="pdb")
                            nc.vector.match_replace(out=pdb[:, :], in_to_replace=v0[:, :],
                                                    in_values=pda[:, :], imm_value=NEG)
                            nc.vector.max(out=v1[:, :], in_=pdb[:, :])
                            nc.vector.max_index(out=i1[:, :], in_max=v1[:, :], in_values=pdb[:, :])
                            nc.vector.match_replace(out=pda[:, :], in_to_replace=v1[:, :],
                                                    in_values=pdb[:, :], imm_value=NEG)
                            nc.vector.max(out=v2[:, :], in_=pda[:, :])
                            nc.vector.max_index(out=i2[:, :], in_max=v2[:, :], in_values=pda[:, :])
                            # stage the 20 indices with 3 DMAs (SP engine) so
                            # the DVE does no assembly copies
                            nc.sync.dma_start(stage[b, isl, 0:8], i0[:, :])
                            nc.sync.dma_start(stage[b, isl, 8:16], i1[:, :])
                            nc.sync.dma_start(stage[b, isl, 16:20], i2[:, 0:4])

                    # ---- gather + window-max + v + relu ----
                    def window_max(G, out_slice):
                        nc.vector.reduce_max(
                            out=out_slice,
                            in_=G.rearrange("p (i t) -> p i t", t=K),
                            axis=AX.X)

                    if l < 3:
                        wrap = gpool.tile([128, N * K // 16], u16, tag="wrap")
                        for g in range(8):
                            bsrc = 2 * p + (0 if g < 4 else 1)
                            lin = stage[bsrc].rearrange("i t -> (i t)").rearrange(
                                "(c r) -> r c", r=16)
                            nc.sync.dma_start(wrap[g * 16:(g + 1) * 16, :], lin)
                        Mp = wkpool.tile([128, N], f32, tag="Mp")
                        for gc in range(8):
                            G = gpool.tile([128, 2560], f32, tag="G", bufs=3)
                            if with_gather:
                                nc.gpsimd.ap_gather(
                                    out_ap=G[:, :], in_ap=upair[:, :],
                                    idxs_ap=wrap[:, gc * 160:(gc + 1) * 160].bitcast(i16),
                                    channels=128, num_elems=N, d=1, num_idxs=2560)
                            window_max(G, Mp[:, gc * 128:(gc + 1) * 128])
                        nc.vector.tensor_add(Mp[:, :], Mp[:, :], vpair[:, :])
                        Fnext = fpool.tile([128, N], f32, tag=f"F{p}",
                                           name=f"F{p}_{l}", bufs=2)
                        nc.scalar.activation(Fnext[:, :], Mp[:, :], AF.Relu)
                        FBnext = fpool.tile([64, N], f32, tag=f"FB{p}",
                                            name=f"FB{p}_{l}", bufs=2)
                        nc.sync.dma_start(FBnext[:, :], Fnext[64:128, :])
                        # global max-pool for this layer
                        gp = selpool.tile([128, 1], f32, tag="gp")
                        nc.vector.reduce_max(out=gp[:, :], in_=Fnext[:, :], axis=AX.X)
                        nc.sync.dma_start(pooled_d[2 * p, l * 64:(l + 1) * 64], gp[0:64, :])
                        nc.sync.dma_start(pooled_d[2 * p + 1, l * 64:(l + 1) * 64], gp[64:128, :])
                        F[p], FB[p] = Fnext, FBnext
                    else:
                        for ui in range(2):
                            b = 2 * p + ui
                            wrap = gpool.tile([128, N * K // 16], u16, tag="wrap")
                            lin = stage[b].rearrange("i t -> (i t)").rearrange(
                                "(c r) -> r c", r=16)
                            for g in range(8):
                                nc.sync.dma_start(wrap[g * 16:(g + 1) * 16, :], lin)
                            Mp = wkpool.tile([128, N], f32, tag="Mp")
                            for gc in range(8):
                                G = gpool.tile([128, 2560], f32, tag="G", bufs=3)
                                if with_gather:
                                    nc.gpsimd.ap_gather(
                                        out_ap=G[:, :], in_ap=u4s[ui][:, :],
                                        idxs_ap=wrap[:, gc * 160:(gc + 1) * 160].bitcast(i16),
                                        channels=128, num_elems=N, d=1, num_idxs=2560)
                                window_max(G, Mp[:, gc * 128:(gc + 1) * 128])
                            nc.vector.tensor_add(Mp[:, :], Mp[:, :], v4s[ui][:, :])
                            x4t = wkpool.tile([128, N], f32, tag="x4t")
                            nc.scalar.activation(x4t[:, :], Mp[:, :], AF.Relu)
                            gp = selpool.tile([128, 1], f32, tag="gp")
                            nc.vector.reduce_max(out=gp[:, :], in_=x4t[:, :], axis=AX.X)
                            nc.sync.dma_start(pooled_d[b, 192:320], gp[:, :])

        # ================= MLP head (own pool scope) =================
        if not with_mlp:
            with tc.tile_pool(name="stub", bufs=1) as spool:
                so = spool.tile([40, BPC], f32, name="so")
                nc.sync.dma_start(so[:, :], pooled_d[:, 0:40].rearrange("b p -> p b"))
                nc.sync.dma_start(out_d, so[:, :])
        elif True:
          with (
            tc.tile_pool(name="mlp", bufs=1) as mpool,
            tc.tile_pool(name="mps", bufs=2, space="PSUM") as mpspool,
          ):
            pooledT = mpool.tile([128, 3, BPC], f32, tag="pooledT")
            for kc in range(3):
                kn = 128 if kc < 2 else 64
                nc.sync.dma_start(pooledT[0:kn, kc, :],
                                  pooled_d[:, kc * 128:kc * 128 + kn].rearrange("b p -> p b"))
            w1sb = mpool.tile([128, 3, 1024], f32, tag="w1sb")
            for kc in range(3):
                kn = 128 if kc < 2 else 64
                nc.sync.dma_start(w1sb[0:kn, kc, :], w1t[kc * 128:kc * 128 + kn, :])
            b1sb = mpool.tile([128, 8], f32, tag="b1sb")
            nc.sync.dma_start(b1sb[:, :], b1)
            h1 = mpool.tile([128, 8, BPC], f32, tag="h1")
            for mc in range(8):
                hp = mpspool.tile([128, BPC], f32, tag="acc")
                for kc in range(3):
                    kn = 128 if kc < 2 else 64
                    nc.tensor.matmul(hp[:, :], w1sb[0:kn, kc, mc * 128:(mc + 1) * 128],
                                     pooledT[0:kn, kc, :], start=(kc == 0), stop=(kc == 2))
                nc.scalar.activation(h1[:, mc, :], hp[:, :], AF.Relu,
                                     bias=b1sb[:, mc:mc + 1])
            w2sb = mpool.tile([128, 8, 512], f32, tag="w2sb")
            for kc in range(8):
                nc.sync.dma_start(w2sb[:, kc, :], w2t[kc * 128:(kc + 1) * 128, :])
            b2sb = mpool.tile([128, 4], f32, tag="b2sb")
            nc.sync.dma_start(b2sb[:, :], b2)
            h2 = mpool.tile([128, 4, BPC], f32, tag="h2")
            for mc in range(4):
                hp = mpspool.tile([128, BPC], f32, tag="acc")
                for kc in range(8):
                    nc.tensor.matmul(hp[:, :], w2sb[:, kc, mc * 128:(mc + 1) * 128],
                                     h1[:, kc, :], start=(kc == 0), stop=(kc == 7))
                nc.scalar.activation(h2[:, mc, :], hp[:, :], AF.Relu,
                                     bias=b2sb[:, mc:mc + 1])
            w3sb = mpool.tile([128, 4, 40], f32, tag="w3sb")
            for kc in range(4):
                nc.sync.dma_start(w3sb[:, kc, :], w3t[kc * 128:(kc + 1) * 128, :])
            b3sb = mpool.tile([40, 1], f32, tag="b3sb")
            nc.sync.dma_start(b3sb[:, :], b3)
            outp = mpspool.tile([40, BPC], f32, tag="acc")
            for kc in range(4):
                nc.tensor.matmul(outp[:, :], w3sb[:, kc, :], h2[:, kc, :],
                                 start=(kc == 0), stop=(kc == 3))
            outsb = mpool.tile([40, BPC], f32, tag="outsb")
            nc.scalar.activation(outsb[:, :], outp[:, :], AF.Identity, bias=b3sb[:, :])
            nc.sync.dma_start(out_d, outsb[:, :])

    nc.compile()
    return nc


# ---------------------------------------------------------------- exec bundle
def _get_bundle(**build_kwargs):
    """Compile the bass program and build the cached 8-core jitted callable."""
    key = ("bundle", tuple(sorted(build_kwargs.items())))
    if key in _CACHE:
        return _CACHE[key]

    import jax
    from jax.sharding import Mesh, PartitionSpec, NamedSharding
    from jax.experimental.shard_map import shard_map
    import concourse.bass2jax as b2j
    from concourse import mybir

    nc = _build_program(**build_kwargs)
    b2j.install_neuronx_cc_hook()

    partition_name = nc.partition_id_tensor.name if nc.partition_id_tensor else None
    in_names, out_names, out_avals, out_shapes = [], [], [], []
    for alloc in nc.m.functions[0].allocations:
        if not isinstance(alloc, mybir.MemoryLocationSet):
            continue
        name = alloc.memorylocations[0].name
        if alloc.kind == "ExternalInput":
            if name != partition_name:
                in_names.append(name)
        elif alloc.kind == "ExternalOutput":
            out_names.append(name)
            shape = tuple(alloc.tensor_shape)
            dtype = mybir.dt.np(alloc.dtype)
            out_avals.append(jax.core.ShapedArray(shape, dtype))
            out_shapes.append((shape, dtype))
    n_params = len(in_names)
    n_outs = len(out_avals)
    in_names_all = in_names + out_names
    if partition_name is not None:
        in_names_all.append(partition_name)
    donate = tuple(range(n_params, n_params + n_outs))

    def _body(*args):
        operands = list(args)
        if partition_name is not None:
            operands.append(b2j.partition_id_tensor())
        outs = b2j._bass_exec_p.bind(
            *operands,
            out_avals=tuple(out_avals),
            in_names=tuple(in_names_all),
            out_names=tuple(out_names),
            lowering_input_output_aliases=(),
            sim_require_finite=True,
            sim_require_nnan=True,
            nc=nc,
        )
        return tuple(outs)

    devices = jax.devices()[:NCORES]
    mesh = Mesh(np.asarray(devices), ("core",))
    sharding = NamedSharding(mesh, PartitionSpec("core"))
    sharded = jax.jit(
        shard_map(_body, mesh=mesh,
                  in_specs=(PartitionSpec("core"),) * (n_params + n_outs),
                  out_specs=(PartitionSpec("core"),) * n_outs,
                  check_rep=False),
        donate_argnums=donate, keep_unused=True)

    bundle = dict(nc=nc, sharded=sharded, in_names=in_names,
                  out_names=out_names, out_shapes=out_shapes,
                  sharding=sharding, n_params=n_params)
    _CACHE[key] = bundle
    return bundle


def _get_device_weights(bundle, inputs):
    """Device-resident concatenated weight buffers, verified per call."""
    import jax
    raw = {k: np.ascontiguousarray(np.asarray(inputs[k], np.float32))
           for k in _WNAMES}
    cached = _CACHE.get("wdev")
    if cached is not None:
        ok = all(np.array_equal(raw[k], cached["raw"][k]) for k in _WNAMES)
        if ok:
            return cached["dev"]
    w = _prep_weights(raw)
    dev = {}
    for name, arr in w.items():
        cat = np.concatenate([arr] * NCORES, axis=0)
        dev[name] = jax.device_put(cat, bundle["sharding"])
    jax.block_until_ready(list(dev.values()))
    _CACHE["wdev"] = dict(raw=raw, dev=dev)
    return dev


# ---------------------------------------------------------------- entry point
def _run(inputs, **spmd_kwargs):
    if spmd_kwargs:
        # debug/trace path: fall back to the uncached spmd runner
        return _run_uncached(inputs, **spmd_kwargs)
    import jax
    bundle = _get_bundle()
    dev_w = _get_device_weights(bundle, inputs)

    x = np.asarray(inputs['x'], dtype=np.float32)   # (32, 1024, 3)
    xT = np.ascontiguousarray(x.reshape(NCORES * BPC, N, 3).transpose(0, 2, 1))

    args = []
    for name in bundle["in_names"]:
        if name == "xT":
            args.append(xT)
        else:
            args.append(dev_w[name])
    for shape, dtype in bundle["out_shapes"]:
        args.append(np.zeros((NCORES * shape[0], *shape[1:]), dtype))

    outs = bundle["sharded"](*args)
    out0 = np.asarray(outs[bundle["out_names"].index("out")])  # (8*40, BPC)
    out = out0.reshape(NCORES, 40, BPC).transpose(0, 2, 1).reshape(B, 40)
    return out.astype(np.float32), _Result()


class _Result:
    exec_time_ns = None
    instructions_and_trace = None


def _run_uncached(inputs, **spmd_kwargs):
    key = "prog"
    if key not in _CACHE:
        _CACHE[key] = _build_program()
    nc = _CACHE[key]

    inputs = {k: np.asarray(v) for k, v in inputs.items()}
    w = _prep_weights(inputs)
    x = np.asarray(inputs['x'], dtype=np.float32)   # (32, 1024, 3)
    in_maps = []
    for c in range(NCORES):
        xs = x[c * BPC:(c + 1) * BPC]                       # (4, 1024, 3)
        m = {'xT': np.ascontiguousarray(xs.transpose(0, 2, 1)).astype(np.float32)}
        m.update({k: np.ascontiguousarray(v) for k, v in w.items()})
        in_maps.append(m)

    from concourse.bass_utils import run_bass_kernel_spmd
    res = run_bass_kernel_spmd(nc, in_maps, core_ids=list(range(NCORES)), **spmd_kwargs)
    out = np.concatenate([r['out'].T for r in res.results], axis=0)  # (32, 40)
    return out.astype(np.float32), res


def kernel(**inputs):
    return _run(inputs)[0]


# revision 8
# speedup vs baseline: 17.7944x; 1.0296x over previous
"""DGCNN forward on 8 Trainium2 NeuronCores, data-parallel over batch.

Contract: kernel(**inputs) takes the FULL (unsharded) inputs from
reference.setup_inputs() and returns the FULL (32, 40) output.

Algorithm (exact, fp32):
  EdgeConv(x)_i = max_{j in knn20(i)} relu(bn(W @ [x_j - x_i; x_i]))
 decomposes (relu/max commute, bn is affine) into
  u_j = s*(wA @ x_j);  v_i = s*((wB-wA) @ x_i) + b
  out_i = relu( max_{j in knn20(i)} u_j  +  v_i )
 so each layer is: pairwise-distance matmul (PE) -> exact top-20 row
 selection (DVE max8/match_replace/max_index) -> gather u rows by index
 (GPSIMD ap_gather) -> windowed max (DVE reduce) -> +v, relu (ACT).

Host path: the compiled SPMD executable, the sharding mesh, and the
device-resident weight buffers are all cached across calls; a call only
ships the point cloud x (48 KiB/core), launches, and fetches the (40,
BPC) logits per core.  Weight inputs are verified against the cached
copy (exact bytewise compare) and re-uploaded if they changed.
"""

import numpy as np

B, N, K = 32, 1024, 20
EPS = 1e-5
NCORES = 8
BPC = B // NCORES          # batches per core
NEG = -1e30

_CACHE = {}

_WNAMES = ['w1', 'g1', 'b1', 'w2', 'g2', 'b2', 'w3', 'g3', 'b3',
           'w4', 'g4', 'b4', 'lw1', 'lb1', 'g5', 'b5', 'lw2', 'lb2',
           'g6', 'b6', 'lw3', 'lb3']


# ---------------------------------------------------------------- weight prep
def _prep_weights(inp):
    """Fold BN into the edge-conv and MLP weights (numpy, host-side)."""
    w = {}
    couts = [64, 64, 64, 128]
    cins = [3, 64, 64, 64]
    for l in range(4):
        wl = inp[f'w{l+1}']            # (Cout, 2C)
        g = inp[f'g{l+1}']
        b = inp[f'b{l+1}']
        C = cins[l]
        s = g / np.sqrt(1.0 + EPS)
        wA = wl[:, :C]                  # acts on (x_j - x_i)
        wB = wl[:, C:]                  # acts on x_i
        Wu = (s[:, None] * wA).T.astype(np.float32)           # (C, Cout)
        Wv = (s[:, None] * (wB - wA)).T.astype(np.float32)    # (C, Cout)
        cout = couts[l]
        if l < 3:
            # batch-pair packing: [Wu | 0] and [0 | Wu], (C, 128)
            zu = np.zeros((C, 64), np.float32)
            w[f'wu{l}a'] = np.concatenate([Wu, zu], 1)
            w[f'wu{l}b'] = np.concatenate([zu, Wu], 1)
            w[f'wv{l}a'] = np.concatenate([Wv, zu], 1)
            w[f'wv{l}b'] = np.concatenate([zu, Wv], 1)
            w[f'bv{l}'] = np.concatenate([b, b]).reshape(128, 1).astype(np.float32)
        else:
            # layer 4: the gather pulls the 64-ch input features pair-packed
            # ([x_A; x_B] on 128 partitions); u4 is rebuilt post-gather with
            # zero-padded weights so each unit's matmul reads only its half.
            z64 = np.zeros((64, 128), np.float32)
            w['wu3a'] = np.concatenate([Wu, z64], 0)   # (128, 128)
            w['wu3b'] = np.concatenate([z64, Wu], 0)
            w[f'wv{l}'] = Wv
            w[f'bv{l}'] = b.reshape(128, 1).astype(np.float32)
    s5 = inp['g5'] / np.sqrt(1.0 + EPS)
    w['w1t'] = (s5[:, None] * inp['lw1']).T.astype(np.float32)      # (320, 1024)
    w['b1'] = (s5 * inp['lb1'] + inp['b5']).reshape(8, 128).T.astype(np.float32).copy()  # (128, 8)
    s6 = inp['g6'] / np.sqrt(1.0 + EPS)
    w['w2t'] = (s6[:, None] * inp['lw2']).T.astype(np.float32)      # (1024, 512)
    w['b2'] = (s6 * inp['lb2'] + inp['b6']).reshape(4, 128).T.astype(np.float32).copy()  # (128, 4)
    w['w3t'] = inp['lw3'].T.astype(np.float32)                      # (512, 40)
    w['b3'] = inp['lb3'].reshape(40, 1).astype(np.float32)
    return w


# ---------------------------------------------------------------- bass program
def _build_program(n_layers=4, with_mlp=True, with_gather=True):
    import concourse.bass as bass
    import concourse.bacc as bacc
    import concourse.mybir as mybir
    from concourse.tile import TileContext

    f32 = mybir.dt.float32
    u16 = mybir.dt.uint16
    i16 = mybir.dt.int16
    AF = mybir.ActivationFunctionType
    AX = mybir.AxisListType

    nc = bacc.Bacc("TRN2")

    # ---- DRAM tensors (per-core inputs) ----
    xT = nc.dram_tensor("xT", [BPC, 3, N], f32, kind="ExternalInput").ap()
    cins = [3, 64, 64, 64]
    couts = [64, 64, 64, 128]
    wt = {}
    for l in range(3):
        for nm in ('wua', 'wub', 'wva', 'wvb'):
            key = f'{nm[:2]}{l}{nm[2]}'
            wt[key] = nc.dram_tensor(key, [cins[l], 128], f32, kind="ExternalInput").ap()
        wt[f'bv{l}'] = nc.dram_tensor(f'bv{l}', [128, 1], f32, kind="ExternalInput").ap()
    wt['wu3'] = nc.dram_tensor('wu3', [64, 128], f32, kind="ExternalInput").ap()
    wt['wv3'] = nc.dram_tensor('wv3', [64, 128], f32, kind="ExternalInput").ap()
    wt['bv3'] = nc.dram_tensor('bv3', [128, 1], f32, kind="ExternalInput").ap()
    w1t = nc.dram_tensor("w1t", [320, 1024], f32, kind="ExternalInput").ap()
    b1 = nc.dram_tensor("b1", [128, 8], f32, kind="ExternalInput").ap()
    w2t = nc.dram_tensor("w2t", [1024, 512], f32, kind="ExternalInput").ap()
    b2 = nc.dram_tensor("b2", [128, 4], f32, kind="ExternalInput").ap()
    w3t = nc.dram_tensor("w3t", [512, 40], f32, kind="ExternalInput").ap()
    b3 = nc.dram_tensor("b3", [40, 1], f32, kind="ExternalInput").ap()

    out_d = nc.dram_tensor("out", [40, BPC], f32, kind="ExternalOutput").ap()
    stage = nc.dram_tensor("idx_stage", [BPC, N, K], u16, kind="Internal").ap()
    pooled_d = nc.dram_tensor("pooled_stage", [BPC, 320], f32, kind="Internal").ap()

    NPAIR = BPC // 2

    with TileContext(nc) as tc:
        with (
            tc.tile_pool(name="const", bufs=1) as cpool,
            tc.tile_pool(name="wpool", bufs=1) as wpool,
            tc.tile_pool(name="feat", bufs=1) as fpool,
            tc.tile_pool(name="work", bufs=2) as wkpool,
            tc.tile_pool(name="uv4", bufs=1) as uv4pool,
            tc.tile_pool(name="pdp", bufs=3) as pdpool,
            tc.tile_pool(name="sel", bufs=6) as selpool,
            tc.tile_pool(name="gath", bufs=2) as gpool,
            tc.tile_pool(name="ps", bufs=2, space="PSUM") as pspool,
            tc.tile_pool(name="psx", bufs=1, space="PSUM") as psxpool,
        ):
            ones_col = cpool.tile([128, 1], f32, tag="onesc")
            nc.vector.memset(ones_col[:, :], 1.0)
            ones_row = cpool.tile([1, N], f32, tag="onesr")
            nc.vector.memset(ones_row[:, :], 1.0)

            # load weights (all at base partition 0 — the PE requires matmul
            # operands to share a base partition, and mixing tile_positions
            # inside one PSUM accumulation group faults on HW)
            wsb = {}
            for l in range(3):
                for key in (f'wu{l}a', f'wu{l}b', f'wv{l}a', f'wv{l}b'):
                    t = wpool.tile([cins[l], 128], f32, tag=key, name=key)
                    nc.sync.dma_start(t[:, :], wt[key])
                    wsb[key] = t
                t = wpool.tile([128, 1], f32, tag=f'bv{l}', name=f'bv{l}')
                nc.sync.dma_start(t[:, :], wt[f'bv{l}'])
                wsb[f'bv{l}'] = t
            for key in ('wu3', 'wv3'):
                t = wpool.tile([64, 128], f32, tag=key, name=key)
                nc.sync.dma_start(t[:, :], wt[key])
                wsb[key] = t
            t = wpool.tile([128, 1], f32, tag='bv3', name='bv3')
            nc.sync.dma_start(t[:, :], wt['bv3'])
            wsb['bv3'] = t

            # Feature state per pair: paired tile F[p] (128, N) holds unit A
            # in partitions [0:64); FB[p] (64, N) is unit B's copy at base 0
            # (extracted by DMA) so every matmul operand starts at partition 0.
            F = [fpool.tile([128, N], f32, tag=f"F{p}", name=f"F{p}", bufs=2)
                 for p in range(NPAIR)]
            FB = [fpool.tile([64, N], f32, tag=f"FB{p}", name=f"FB{p}", bufs=2)
                  for p in range(NPAIR)]
            for p in range(NPAIR):
                nc.sync.dma_start(F[p][0:3, :], xT[2 * p, :, :])
                nc.sync.dma_start(FB[p][0:3, :], xT[2 * p + 1, :, :])

            def window_max(G, out_slice):
                nc.vector.reduce_max(
                    out=out_slice,
                    in_=G.rearrange("p (i t) -> p i t", t=K),
                    axis=AX.X)

            NI = 5120           # indices per gather call (chunk of 256 rows)
            NGC = N * K // NI   # gather calls per index stream

            for l in range(n_layers):
                C = cins[l]
                uvt = {}
                # ======== phase 1: u/v tables + pd + top-20 selection ========
                # (emitted for every pair before any gather so the DVE's
                # selection work overlaps the GPSIMD gathers in phase 2)
                for p in range(NPAIR):
                    Fp = F[p]
                    FBp = FB[p]
                    funits = (Fp, FBp)  # unit -> feature AP source (base 0)
                    # ---- squared norms (per unit, base partition 0) ----
                    negxx = [None, None]
                    for ui in range(2):
                        fsq = wkpool.tile([64, N], f32, tag=f"fsq{ui}",
                                          name=f"fsq{ui}", bufs=1)
                        nc.scalar.activation(fsq[0:C, :], funits[ui][0:C, :], AF.Square)
                        xxp = psxpool.tile([1, N], f32, tag="xx", name="xxp")
                        for h in range(2):
                            sl = slice(h * 512, (h + 1) * 512)
                            nc.tensor.matmul(xxp[:, sl], ones_col[0:C, :],
                                             fsq[0:C, sl], start=True, stop=True)
                        nxx = wkpool.tile([1, N], f32, tag=f"nxx{ui}", name=f"nxx{ui}")
                        nc.scalar.activation(nxx[:, :], xxp[:, :], AF.Copy, scale=-1.0)
                        negxx[ui] = nxx

                    # ---- u/v feature tables ----
                    if l < 3:
                        # batch-pair packed: psum = [u_A ; u_B] via padded weights
                        upair = wkpool.tile([128, N], f32, tag="upair", name=f"up{p}")
                        vpair = wkpool.tile([128, N], f32, tag="vpair", name=f"vp{p}")
                        for h in range(2):
                            sl = slice(h * 512, (h + 1) * 512)
                            up = pspool.tile([128, 512], f32, tag="acc")
                            vp = pspool.tile([128, 512], f32, tag="acc")
                            nc.tensor.matmul(up[:, :], wsb[f'wu{l}a'][:, :], Fp[0:C, sl],
                                             start=True, stop=False)
                            nc.tensor.matmul(up[:, :], wsb[f'wu{l}b'][:, :],
                                             FBp[0:C, sl], start=False, stop=True)
                            nc.tensor.matmul(vp[:, :], wsb[f'wv{l}a'][:, :], Fp[0:C, sl],
                                             start=True, stop=False)
                            nc.tensor.matmul(vp[:, :], wsb[f'wv{l}b'][:, :],
                                             FBp[0:C, sl], start=False, stop=True)
                            nc.scalar.activation(upair[:, sl], up[:, :], AF.Copy)
                            nc.scalar.activation(vpair[:, sl], vp[:, :], AF.Identity,
                                                 bias=wsb[f'bv{l}'][:, :])
                            del up, vp
                        uvt[p] = (upair, vpair)
                    else:
                        # layer 4: Cout=128 -> per-unit full-width tables
                        u4s, v4s = [], []
                        for ui in range(2):
                            b = 2 * p + ui
                            u4 = uv4pool.tile([128, N], f32, tag=f"u4_{b}",
                                              name=f"u4_{b}")
                            v4 = uv4pool.tile([128, N], f32, tag=f"v4_{b}",
                                              name=f"v4_{b}")
                            for h in range(2):
                                sl = slice(h * 512, (h + 1) * 512)
                                up = pspool.tile([128, 512], f32, tag="acc")
                                vp = pspool.tile([128, 512], f32, tag="acc")
                                nc.tensor.matmul(up[:, :], wsb['wu3'][:, :],
                                                 funits[ui][0:C, sl], start=True, stop=True)
                                nc.tensor.matmul(vp[:, :], wsb['wv3'][:, :],
                                                 funits[ui][0:C, sl], start=True, stop=True)
                                nc.scalar.activation(u4[:, sl], up[:, :], AF.Copy)
                                nc.scalar.activation(v4[:, sl], vp[:, :], AF.Identity,
                                                     bias=wsb['bv3'][:, :])
                                del up, vp
                            u4s.append(u4)
                            v4s.append(v4)
                        uvt[p] = (u4s, v4s)

                    for ui in range(2):
                        b = 2 * p + ui
                        FX = funits[ui]
                        # ---- pd + top-20 selection per 128-row chunk ----
                        for ic in range(8):
                            isl = slice(ic * 128, (ic + 1) * 128)
                            pdp = pspool.tile([128, 1024], f32, tag="pd")
                            for h in range(2):
                                sl = slice(h * 512, (h + 1) * 512)
                                nc.tensor.matmul(pdp[:, sl], FX[0:C, isl],
                                                 FX[0:C, sl], start=True, stop=False)
                                nc.tensor.matmul(pdp[:, sl], FX[0:C, isl],
                                                 FX[0:C, sl], start=False, stop=False)
                                nc.tensor.matmul(pdp[:, sl], negxx[ui][:, isl],
                                                 ones_row[:, sl], start=False, stop=False)
                                nc.tensor.matmul(pdp[:, sl], ones_row[:, isl],
                                                 negxx[ui][:, sl], start=False, stop=True)
                            pda = pdpool.tile([128, 1024], f32, tag="pda")
                            nc.scalar.activation(pda[:, :], pdp[:, :], AF.Copy)
                            del pdp

                            v0 = selpool.tile([128, 8], f32, tag="v0")
                            v1 = selpool.tile([128, 8], f32, tag="v1")
                            v2 = selpool.tile([128, 8], f32, tag="v2")
                            iv = selpool.tile([128, 24], u16, tag="iv")
                            nc.vector.max(out=v0[:, :], in_=pda[:, :])
                            nc.vector.max_index(out=iv[:, 0:8], in_max=v0[:, :], in_values=pda[:, :])
                            pdb = pdpool.tile([128, 1024], f32, tag="pdb")
                            nc.vector.match_replace(out=pdb[:, :], in_to_replace=v0[:, :],
                                                    in_values=pda[:, :], imm_value=NEG)
                            nc.vector.max(out=v1[:, :], in_=pdb[:, :])
                            nc.vector.max_index(out=iv[:, 8:16], in_max=v1[:, :], in_values=pdb[:, :])
                            nc.vector.match_replace(out=pda[:, :], in_to_replace=v1[:, :],
                                                    in_values=pdb[:, :], imm_value=NEG)
                            nc.vector.max(out=v2[:, :], in_=pda[:, :])
                            nc.vector.max_index(out=iv[:, 16:24], in_max=v2[:, :], in_values=pda[:, :])
                            # stage the 20 indices with one DMA (SP engine)
                            nc.sync.dma_start(stage[b, isl, 0:20], iv[:, 0:20])

                # ======== phase 2: gather + window-max + v + relu + pool ========
                for p in range(NPAIR):
                    if l < 3:
                        upair, vpair = uvt[p]
                        wrap = gpool.tile([128, N * K // 16], u16, tag="wrap")
                        for g in range(8):
                            bsrc = 2 * p + (0 if g < 4 else 1)
                            lin = stage[bsrc].rearrange("i t -> (i t)").rearrange(
                                "(c r) -> r c", r=16)
                            nc.sync.dma_start(wrap[g * 16:(g + 1) * 16, :], lin)
                        Mp = wkpool.tile([128, N], f32, tag="Mp")
                        NW = NI // K
                        for gc in range(NGC):
                            G = gpool.tile([128, NI], f32, tag="G", bufs=2)
                            if with_gather:
                                nc.gpsimd.ap_gather(
                                    out_ap=G[:, :], in_ap=upair[:, :],
                                    idxs_ap=wrap[:, gc * (NI // 16):(gc + 1) * (NI // 16)].bitcast(i16),
                                    channels=128, num_elems=N, d=1, num_idxs=NI)
                            else:
                                nc.vector.memset(G[:, :], 0.0)
                            window_max(G, Mp[:, gc * NW:(gc + 1) * NW])
                        nc.vector.tensor_add(Mp[:, :], Mp[:, :], vpair[:, :])
                        Fnext = fpool.tile([128, N], f32, tag=f"F{p}",
                                           name=f"F{p}_{l}", bufs=2)
                        nc.scalar.activation(Fnext[:, :], Mp[:, :], AF.Relu)
                        FBnext = fpool.tile([64, N], f32, tag=f"FB{p}",
                                            name=f"FB{p}_{l}", bufs=2)
                        nc.sync.dma_start(FBnext[:, :], Fnext[64:128, :])
                        # global max-pool for this layer
                        gp = selpool.tile([128, 1], f32, tag="gp")
                        nc.vector.reduce_max(out=gp[:, :], in_=Fnext[:, :], axis=AX.X)
                        nc.sync.dma_start(pooled_d[2 * p, l * 64:(l + 1) * 64], gp[0:64, :])
                        nc.sync.dma_start(pooled_d[2 * p + 1, l * 64:(l + 1) * 64], gp[64:128, :])
                        F[p], FB[p] = Fnext, FBnext
                    else:
                        u4s, v4s = uvt[p]
                        for ui in range(2):
                            b = 2 * p + ui
                            wrap = gpool.tile([128, N * K // 16], u16, tag="wrap")
                            lin = stage[b].rearrange("i t -> (i t)").rearrange(
                                "(c r) -> r c", r=16)
                            for g in range(8):
                                nc.sync.dma_start(wrap[g * 16:(g + 1) * 16, :], lin)
                            Mp = wkpool.tile([128, N], f32, tag="Mp")
                            NW = NI // K
                            for gc in range(NGC):
                                G = gpool.tile([128, NI], f32, tag="G", bufs=2)
                                if with_gather:
                                    nc.gpsimd.ap_gather(
                                        out_ap=G[:, :], in_ap=u4s[ui][:, :],
                                        idxs_ap=wrap[:, gc * (NI // 16):(gc + 1) * (NI // 16)].bitcast(i16),
                                        channels=128, num_elems=N, d=1, num_idxs=NI)
                                else:
                                    nc.vector.memset(G[:, :], 0.0)
                                window_max(G, Mp[:, gc * NW:(gc + 1) * NW])
                            nc.vector.tensor_add(Mp[:, :], Mp[:, :], v4s[ui][:, :])
                            x4t = wkpool.tile([128, N], f32, tag="x4t")
                            nc.scalar.activation(x4t[:, :], Mp[:, :], AF.Relu)
                            gp = selpool.tile([128, 1], f32, tag="gp")
                            nc.vector.reduce_max(out=gp[:, :], in_=x4t[:, :], axis=AX.X)
                            nc.sync.dma_start(pooled_d[b, 192:320], gp[:, :])

        # ================= MLP head (own pool scope) =================
        if not with_mlp:
            with tc.tile_pool(name="stub", bufs=1) as spool:
                so = spool.tile([40, BPC], f32, name="so")
                nc.sync.dma_start(so[:, :], pooled_d[:, 0:40].rearrange("b p -> p b"))
                nc.sync.dma_start(out_d, so[:, :])
        elif True:
          with (
            tc.tile_pool(name="mlp", bufs=1) as mpool,
            tc.tile_pool(name="mps", bufs=2, space="PSUM") as mpspool,
          ):
            pooledT = mpool.tile([128, 3, BPC], f32, tag="pooledT")
            for kc in range(3):
                kn = 128 if kc < 2 else 64
                nc.sync.dma_start(pooledT[0:kn, kc, :],
                                  pooled_d[:, kc * 128:kc * 128 + kn].rearrange("b p -> p b"))
            w1sb = mpool.tile([128, 3, 1024], f32, tag="w1sb")
            for kc in range(3):
                kn = 128 if kc < 2 else 64
                nc.sync.dma_start(w1sb[0:kn, kc, :], w1t[kc * 128:kc * 128 + kn, :])
            b1sb = mpool.tile([128, 8], f32, tag="b1sb")
            nc.sync.dma_start(b1sb[:, :], b1)
            h1 = mpool.tile([128, 8, BPC], f32, tag="h1")
            for mc in range(8):
                hp = mpspool.tile([128, BPC], f32, tag="acc")
                for kc in range(3):
                    kn = 128 if kc < 2 else 64
                    nc.tensor.matmul(hp[:, :], w1sb[0:kn, kc, mc * 128:(mc + 1) * 128],
                                     pooledT[0:kn, kc, :], start=(kc == 0), stop=(kc == 2))
                nc.scalar.activation(h1[:, mc, :], hp[:, :], AF.Relu,
                                     bias=b1sb[:, mc:mc + 1])
            w2sb = mpool.tile([128, 8, 512], f32, tag="w2sb")
            for kc in range(8):
                nc.sync.dma_start(w2sb[:, kc, :], w2t[kc * 128:(kc + 1) * 128, :])
            b2sb = mpool.tile([128, 4], f32, tag="b2sb")
            nc.sync.dma_start(b2sb[:, :], b2)
            h2 = mpool.tile([128, 4, BPC], f32, tag="h2")
            for mc in range(4):
                hp = mpspool.tile([128, BPC], f32, tag="acc")
                for kc in range(8):
                    nc.tensor.matmul(hp[:, :], w2sb[:, kc, mc * 128:(mc + 1) * 128],
                                     h1[:, kc, :], start=(kc == 0), stop=(kc == 7))
                nc.scalar.activation(h2[:, mc, :], hp[:, :], AF.Relu,
                                     bias=b2sb[:, mc:mc + 1])
            w3sb = mpool.tile([128, 4, 40], f32, tag="w3sb")
            for kc in range(4):
                nc.sync.dma_start(w3sb[:, kc, :], w3t[kc * 128:(kc + 1) * 128, :])
            b3sb = mpool.tile([40, 1], f32, tag="b3sb")
            nc.sync.dma_start(b3sb[:, :], b3)
            outp = mpspool.tile([40, BPC], f32, tag="acc")
            for kc in range(4):
                nc.tensor.matmul(outp[:, :], w3sb[:, kc, :], h2[:, kc, :],
                                 start=(kc == 0), stop=(kc == 3))
            outsb = mpool.tile([40, BPC], f32, tag="outsb")
            nc.scalar.activation(outsb[:, :], outp[:, :], AF.Identity, bias=b3sb[:, :])
            nc.sync.dma_start(out_d, outsb[:, :])

    nc.compile()
    return nc


# ---------------------------------------------------------------- exec bundle
def _get_bundle(**build_kwargs):
    """Compile the bass program and build the cached 8-core jitted callable."""
    key = ("bundle", tuple(sorted(build_kwargs.items())))
    if key in _CACHE:
        return _CACHE[key]

    import jax
    from jax.sharding import Mesh, PartitionSpec, NamedSharding
    from jax.experimental.shard_map import shard_map
    import concourse.bass2jax as b2j
    from concourse import mybir

    nc = _build_program(**build_kwargs)
    b2j.install_neuronx_cc_hook()

    partition_name = nc.partition_id_tensor.name if nc.partition_id_tensor else None
    in_names, out_names, out_avals, out_shapes = [], [], [], []
    for alloc in nc.m.functions[0].allocations:
        if not isinstance(alloc, mybir.MemoryLocationSet):
            continue
        name = alloc.memorylocations[0].name
        if alloc.kind == "ExternalInput":
            if name != partition_name:
                in_names.append(name)
        elif alloc.kind == "ExternalOutput":
            out_names.append(name)
            shape = tuple(alloc.tensor_shape)
            dtype = mybir.dt.np(alloc.dtype)
            out_avals.append(jax.core.ShapedArray(shape, dtype))
            out_shapes.append((shape, dtype))
    n_params = len(in_names)
    n_outs = len(out_avals)
    in_names_all = in_names + out_names
    if partition_name is not None:
        in_names_all.append(partition_name)
    donate = tuple(range(n_params, n_params + n_outs))

    def _body(*args):
        operands = list(args)
        if partition_name is not None:
            operands.append(b2j.partition_id_tensor())
        outs = b2j._bass_exec_p.bind(
            *operands,
            out_avals=tuple(out_avals),
            in_names=tuple(in_names_all),
            out_names=tuple(out_names),
            lowering_input_output_aliases=(),
            sim_require_finite=True,
            sim_require_nnan=True,
            nc=nc,
        )
        return tuple(outs)

    devices = jax.devices()[:NCORES]
    mesh = Mesh(np.asarray(devices), ("core",))
    sharding = NamedSharding(mesh, PartitionSpec("core"))
    sharded = jax.jit(
        shard_map(_body, mesh=mesh,
                  in_specs=(PartitionSpec("core"),) * (n_params + n_outs),
                  out_specs=(PartitionSpec("core"),) * n_outs,
                  check_rep=False),
        donate_argnums=donate, keep_unused=True)

    bundle = dict(nc=nc, sharded=sharded, in_names=in_names,
                  out_names=out_names, out_shapes=out_shapes,
                  sharding=sharding, n_params=n_params)
    _CACHE[key] = bundle
    return bundle


def _get_device_weights(bundle, inputs):
    """Device-resident concatenated weight buffers, verified per call."""
    import jax
    raw = {k: np.ascontiguousarray(np.asarray(inputs[k], np.float32))
           for k in _WNAMES}
    cached = _CACHE.get("wdev")
    if cached is not None:
        ok = all(np.array_equal(raw[k], cached["raw"][k]) for k in _WNAMES)
        if ok:
            return cached["dev"]
    w = _prep_weights(raw)
    dev = {}
    for name, arr in w.items():
        cat = np.concatenate([arr] * NCORES, axis=0)
        dev[name] = jax.device_put(cat, bundle["sharding"])
    jax.block_until_ready(list(dev.values()))
    _CACHE["wdev"] = dict(raw=raw, dev=dev)
    return dev


# ---------------------------------------------------------------- entry point
def _run(inputs, **spmd_kwargs):
    if spmd_kwargs:
        # debug/trace path: fall back to the uncached spmd runner
        return _run_uncached(inputs, **spmd_kwargs)
    import jax
    bundle = _get_bundle()
    dev_w = _get_device_weights(bundle, inputs)

    x = np.asarray(inputs['x'], dtype=np.float32)   # (32, 1024, 3)
    xT = np.ascontiguousarray(x.reshape(NCORES * BPC, N, 3).transpose(0, 2, 1))

    args = []
    for name in bundle["in_names"]:
        if name == "xT":
            args.append(xT)
        else:
            args.append(dev_w[name])
    for shape, dtype in bundle["out_shapes"]:
        args.append(np.zeros((NCORES * shape[0], *shape[1:]), dtype))

    outs = bundle["sharded"](*args)
    out0 = np.asarray(outs[bundle["out_names"].index("out")])  # (8*40, BPC)
    out = out0.reshape(NCORES, 40, BPC).transpose(0, 2, 1).reshape(B, 40)
    return out.astype(np.float32), _Result()


class _Result:
    exec_time_ns = None
    instructions_and_trace = None


def _run_uncached(inputs, **spmd_kwargs):
    key = "prog"
    if key not in _CACHE:
        _CACHE[key] = _build_program()
    nc = _CACHE[key]

    inputs = {k: np.asarray(v) for k, v in inputs.items()}
    w = _prep_weights(inputs)
    x = np.asarray(inputs['x'], dtype=np.float32)   # (32, 1024, 3)
    in_maps = []
    for c in range(NCORES):
        xs = x[c * BPC:(c + 1) * BPC]                       # (4, 1024, 3)
        m = {'xT': np.ascontiguousarray(xs.transpose(0, 2, 1)).astype(np.float32)}
        m.update({k: np.ascontiguousarray(v) for k, v in w.items()})
        in_maps.append(m)

    from concourse.bass_utils import run_bass_kernel_spmd
    res = run_bass_kernel_spmd(nc, in_maps, core_ids=list(range(NCORES)), **spmd_kwargs)
    out = np.concatenate([r['out'].T for r in res.results], axis=0)  # (32, 40)
    return out.astype(np.float32), res


def kernel(**inputs):
    return _run(inputs)[0]


# revision 13
# speedup vs baseline: 20.5236x; 1.1534x over previous
"""DGCNN forward on 8 Trainium2 NeuronCores, data-parallel over batch.

Contract: kernel(**inputs) takes the FULL (unsharded) inputs from
reference.setup_inputs() and returns the FULL (32, 40) output.

Algorithm (exact, fp32):
  EdgeConv(x)_i = max_{j in knn20(i)} relu(bn(W @ [x_j - x_i; x_i]))
 decomposes (relu/max commute, bn is affine) into
  u_j = s*(wA @ x_j);  v_i = s*((wB-wA) @ x_i) + b
  out_i = relu( max_{j in knn20(i)} u_j  +  v_i )
 so each layer is: pairwise-distance matmul (PE) -> exact top-20 row
 selection (DVE max8/match_replace/max_index) -> gather u rows by index
 (GPSIMD ap_gather) -> windowed max (DVE reduce) -> +v, relu (ACT).

Host path: the compiled SPMD executable, the sharding mesh, and the
device-resident weight buffers are all cached across calls; a call only
ships the point cloud x (48 KiB/core), launches, and fetches the (40,
BPC) logits per core.  Weight inputs are verified against the cached
copy (exact bytewise compare) and re-uploaded if they changed.
"""

import numpy as np

B, N, K = 32, 1024, 20
EPS = 1e-5
NCORES = 8
BPC = B // NCORES          # batches per core
NEG = -1e30

_CACHE = {}

_WNAMES = ['w1', 'g1', 'b1', 'w2', 'g2', 'b2', 'w3', 'g3', 'b3',
           'w4', 'g4', 'b4', 'lw1', 'lb1', 'g5', 'b5', 'lw2', 'lb2',
           'g6', 'b6', 'lw3', 'lb3']


# ---------------------------------------------------------------- weight prep
def _prep_weights(inp):
    """Fold BN into the edge-conv and MLP weights (numpy, host-side)."""
    w = {}
    couts = [64, 64, 64, 128]
    cins = [3, 64, 64, 64]
    for l in range(4):
        wl = inp[f'w{l+1}']            # (Cout, 2C)
        g = inp[f'g{l+1}']
        b = inp[f'b{l+1}']
        C = cins[l]
        s = g / np.sqrt(1.0 + EPS)
        wA = wl[:, :C]                  # acts on (x_j - x_i)
        wB = wl[:, C:]                  # acts on x_i
        Wu = (s[:, None] * wA).T.astype(np.float32)           # (C, Cout)
        Wv = (s[:, None] * (wB - wA)).T.astype(np.float32)    # (C, Cout)
        cout = couts[l]
        if l < 3:
            # batch-pair packing: [Wu | 0] and [0 | Wu], (C, 128)
            zu = np.zeros((C, 64), np.float32)
            w[f'wu{l}a'] = np.concatenate([Wu, zu], 1)
            w[f'wu{l}b'] = np.concatenate([zu, Wu], 1)
            w[f'wv{l}a'] = np.concatenate([Wv, zu], 1)
            w[f'wv{l}b'] = np.concatenate([zu, Wv], 1)
            w[f'bv{l}'] = np.concatenate([b, b]).reshape(128, 1).astype(np.float32)
        else:
            # layer 4: the gather pulls the 64-ch input features pair-packed
            # ([x_A; x_B] on 128 partitions); u4 is rebuilt post-gather with
            # zero-padded weights so each unit's matmul reads only its half.
            z64 = np.zeros((64, 128), np.float32)
            w['wu3a'] = np.concatenate([Wu, z64], 0)   # (128, 128)
            w['wu3b'] = np.concatenate([z64, Wu], 0)
            w[f'wv{l}'] = Wv
            w[f'bv{l}'] = b.reshape(128, 1).astype(np.float32)
    s5 = inp['g5'] / np.sqrt(1.0 + EPS)
    w['w1t'] = (s5[:, None] * inp['lw1']).T.astype(np.float32)      # (320, 1024)
    w['b1'] = (s5 * inp['lb1'] + inp['b5']).reshape(8, 128).T.astype(np.float32).copy()  # (128, 8)
    s6 = inp['g6'] / np.sqrt(1.0 + EPS)
    w['w2t'] = (s6[:, None] * inp['lw2']).T.astype(np.float32)      # (1024, 512)
    w['b2'] = (s6 * inp['lb2'] + inp['b6']).reshape(4, 128).T.astype(np.float32).copy()  # (128, 4)
    w['w3t'] = inp['lw3'].T.astype(np.float32)                      # (512, 40)
    w['b3'] = inp['lb3'].reshape(40, 1).astype(np.float32)
    return w


# ---------------------------------------------------------------- bass program
def _build_program(n_layers=4, with_mlp=True, with_gather=True):
    import concourse.bass as bass
    import concourse.bacc as bacc
    import concourse.mybir as mybir
    from concourse.tile import TileContext

    f32 = mybir.dt.float32
    u16 = mybir.dt.uint16
    i16 = mybir.dt.int16
    AF = mybir.ActivationFunctionType
    AX = mybir.AxisListType

    nc = bacc.Bacc("TRN2")

    # ---- DRAM tensors (per-core inputs) ----
    xT = nc.dram_tensor("xT", [BPC, 3, N], f32, kind="ExternalInput").ap()
    cins = [3, 64, 64, 64]
    couts = [64, 64, 64, 128]
    wt = {}
    for l in range(3):
        for nm in ('wua', 'wub', 'wva', 'wvb'):
            key = f'{nm[:2]}{l}{nm[2]}'
            wt[key] = nc.dram_tensor(key, [cins[l], 128], f32, kind="ExternalInput").ap()
        wt[f'bv{l}'] = nc.dram_tensor(f'bv{l}', [128, 1], f32, kind="ExternalInput").ap()
    wt['wu3a'] = nc.dram_tensor('wu3a', [128, 128], f32, kind="ExternalInput").ap()
    wt['wu3b'] = nc.dram_tensor('wu3b', [128, 128], f32, kind="ExternalInput").ap()
    wt['wv3'] = nc.dram_tensor('wv3', [64, 128], f32, kind="ExternalInput").ap()
    wt['bv3'] = nc.dram_tensor('bv3', [128, 1], f32, kind="ExternalInput").ap()
    w1t = nc.dram_tensor("w1t", [320, 1024], f32, kind="ExternalInput").ap()
    b1 = nc.dram_tensor("b1", [128, 8], f32, kind="ExternalInput").ap()
    w2t = nc.dram_tensor("w2t", [1024, 512], f32, kind="ExternalInput").ap()
    b2 = nc.dram_tensor("b2", [128, 4], f32, kind="ExternalInput").ap()
    w3t = nc.dram_tensor("w3t", [512, 40], f32, kind="ExternalInput").ap()
    b3 = nc.dram_tensor("b3", [40, 1], f32, kind="ExternalInput").ap()

    out_d = nc.dram_tensor("out", [40, BPC], f32, kind="ExternalOutput").ap()
    stage = nc.dram_tensor("idx_stage", [BPC, N, K], u16, kind="Internal").ap()
    pooled_d = nc.dram_tensor("pooled_stage", [BPC, 320], f32, kind="Internal").ap()

    NPAIR = BPC // 2

    with TileContext(nc) as tc:
        with (
            tc.tile_pool(name="const", bufs=1) as cpool,
            tc.tile_pool(name="wpool", bufs=1) as wpool,
            tc.tile_pool(name="feat", bufs=1) as fpool,
            tc.tile_pool(name="work", bufs=2) as wkpool,
            tc.tile_pool(name="uv4", bufs=1) as uv4pool,
            tc.tile_pool(name="pdp", bufs=3) as pdpool,
            tc.tile_pool(name="sel", bufs=6) as selpool,
            tc.tile_pool(name="gath", bufs=2) as gpool,
            tc.tile_pool(name="ps", bufs=2, space="PSUM") as pspool,
            tc.tile_pool(name="psx", bufs=1, space="PSUM") as psxpool,
        ):
            ones_col = cpool.tile([128, 1], f32, tag="onesc")
            nc.vector.memset(ones_col[:, :], 1.0)
            ones_row = cpool.tile([1, N], f32, tag="onesr")
            nc.vector.memset(ones_row[:, :], 1.0)

            # load weights (all at base partition 0 — the PE requires matmul
            # operands to share a base partition, and mixing tile_positions
            # inside one PSUM accumulation group faults on HW)
            wsb = {}
            for l in range(3):
                for key in (f'wu{l}a', f'wu{l}b', f'wv{l}a', f'wv{l}b'):
                    t = wpool.tile([cins[l], 128], f32, tag=key, name=key)
                    nc.sync.dma_start(t[:, :], wt[key])
                    wsb[key] = t
                t = wpool.tile([128, 1], f32, tag=f'bv{l}', name=f'bv{l}')
                nc.sync.dma_start(t[:, :], wt[f'bv{l}'])
                wsb[f'bv{l}'] = t
            for key in ('wu3a', 'wu3b'):
                t = wpool.tile([128, 128], f32, tag=key, name=key)
                nc.sync.dma_start(t[:, :], wt[key])
                wsb[key] = t
            t = wpool.tile([64, 128], f32, tag='wv3', name='wv3')
            nc.sync.dma_start(t[:, :], wt['wv3'])
            wsb['wv3'] = t
            t = wpool.tile([128, 1], f32, tag='bv3', name='bv3')
            nc.sync.dma_start(t[:, :], wt['bv3'])
            wsb['bv3'] = t

            # Feature state per pair: paired tile F[p] (128, N) holds unit A
            # in partitions [0:64); FB[p] (64, N) is unit B's copy at base 0
            # (extracted by DMA) so every matmul operand starts at partition 0.
            F = [fpool.tile([128, N], f32, tag=f"F{p}", name=f"F{p}", bufs=2)
                 for p in range(NPAIR)]
            FB = [fpool.tile([64, N], f32, tag=f"FB{p}", name=f"FB{p}", bufs=2)
                  for p in range(NPAIR)]
            for p in range(NPAIR):
                nc.sync.dma_start(F[p][0:3, :], xT[2 * p, :, :])
                nc.sync.dma_start(FB[p][0:3, :], xT[2 * p + 1, :, :])

            def window_max(G, out_slice):
                nc.vector.reduce_max(
                    out=out_slice,
                    in_=G.rearrange("p (i t) -> p i t", t=K),
                    axis=AX.X)

            NI = 5120           # indices per gather call (chunk of 256 rows)
            NGC = N * K // NI   # gather calls per index stream

            for l in range(n_layers):
                C = cins[l]
                uvt = {}
                # ======== phase 1: u/v tables + pd + top-20 selection ========
                # (emitted for every pair before any gather so the DVE's
                # selection work overlaps the GPSIMD gathers in phase 2)
                for p in range(NPAIR):
                    Fp = F[p]
                    FBp = FB[p]
                    funits = (Fp, FBp)  # unit -> feature AP source (base 0)
                    # ---- squared norms (per unit, base partition 0) ----
                    negxx = [None, None]
                    for ui in range(2):
                        fsq = wkpool.tile([64, N], f32, tag=f"fsq{ui}",
                                          name=f"fsq{ui}", bufs=1)
                        nc.scalar.activation(fsq[0:C, :], funits[ui][0:C, :], AF.Square)
                        xxp = psxpool.tile([1, N], f32, tag="xx", name="xxp")
                        for h in range(2):
                            sl = slice(h * 512, (h + 1) * 512)
                            nc.tensor.matmul(xxp[:, sl], ones_col[0:C, :],
                                             fsq[0:C, sl], start=True, stop=True)
                        nxx = wkpool.tile([1, N], f32, tag=f"nxx{ui}", name=f"nxx{ui}")
                        nc.scalar.activation(nxx[:, :], xxp[:, :], AF.Copy, scale=-1.0)
                        negxx[ui] = nxx

                    # ---- u/v feature tables ----
                    if l < 3:
                        # batch-pair packed: psum = [u_A ; u_B] via padded weights
                        upair = wkpool.tile([128, N], f32, tag="upair", name=f"up{p}")
                        vpair = wkpool.tile([128, N], f32, tag="vpair", name=f"vp{p}")
                        for h in range(2):
                            sl = slice(h * 512, (h + 1) * 512)
                            up = pspool.tile([128, 512], f32, tag="acc", bufs=1)
                            vp = pspool.tile([128, 512], f32, tag="acc", bufs=1)
                            nc.tensor.matmul(up[:, :], wsb[f'wu{l}a'][:, :], Fp[0:C, sl],
                                             start=True, stop=False)
                            nc.tensor.matmul(up[:, :], wsb[f'wu{l}b'][:, :],
                                             FBp[0:C, sl], start=False, stop=True)
                            nc.tensor.matmul(vp[:, :], wsb[f'wv{l}a'][:, :], Fp[0:C, sl],
                                             start=True, stop=False)
                            nc.tensor.matmul(vp[:, :], wsb[f'wv{l}b'][:, :],
                                             FBp[0:C, sl], start=False, stop=True)
                            nc.scalar.activation(upair[:, sl], up[:, :], AF.Copy)
                            nc.scalar.activation(vpair[:, sl], vp[:, :], AF.Identity,
                                                 bias=wsb[f'bv{l}'][:, :])
                            del up, vp
                        uvt[p] = (upair, vpair)
                    else:
                        # layer 4: only the v tables are needed up front — u4
                        # is rebuilt post-gather from the gathered features.
                        v4s = []
                        for ui in range(2):
                            b = 2 * p + ui
                            v4 = uv4pool.tile([128, N], f32, tag=f"v4_{b}",
                                              name=f"v4_{b}")
                            for h in range(2):
                                sl = slice(h * 512, (h + 1) * 512)
                                vp = pspool.tile([128, 512], f32, tag="acc", bufs=1)
                                nc.tensor.matmul(vp[:, :], wsb['wv3'][:, :],
                                                 funits[ui][0:C, sl], start=True, stop=True)
                                nc.scalar.activation(v4[:, sl], vp[:, :], AF.Identity,
                                                     bias=wsb['bv3'][:, :])
                                del vp
                            v4s.append(v4)
                        uvt[p] = v4s

                    for ui in range(2):
                        b = 2 * p + ui
                        FX = funits[ui]
                        # ---- pd + top-20 selection per 128-row chunk ----
                        for ic in range(8):
                            isl = slice(ic * 128, (ic + 1) * 128)
                            pdp = pspool.tile([128, 1024], f32, tag="pd", bufs=1)
                            for h in range(2):
                                sl = slice(h * 512, (h + 1) * 512)
                                nc.tensor.matmul(pdp[:, sl], FX[0:C, isl],
                                                 FX[0:C, sl], start=True, stop=False)
                                nc.tensor.matmul(pdp[:, sl], FX[0:C, isl],
                                                 FX[0:C, sl], start=False, stop=False)
                                nc.tensor.matmul(pdp[:, sl], negxx[ui][:, isl],
                                                 ones_row[:, sl], start=False, stop=False)
                                nc.tensor.matmul(pdp[:, sl], ones_row[:, isl],
                                                 negxx[ui][:, sl], start=False, stop=True)
                            pda = pdpool.tile([128, 1024], f32, tag="pda")
                            nc.scalar.activation(pda[:, :], pdp[:, :], AF.Copy)
                            del pdp

                            v0 = selpool.tile([128, 8], f32, tag="v0")
                            v1 = selpool.tile([128, 8], f32, tag="v1")
                            v2 = selpool.tile([128, 8], f32, tag="v2")
                            iv = selpool.tile([128, 24], u16, tag="iv")
                            nc.vector.max(out=v0[:, :], in_=pda[:, :])
                            nc.vector.max_index(out=iv[:, 0:8], in_max=v0[:, :], in_values=pda[:, :])
                            pdb = pdpool.tile([128, 1024], f32, tag="pdb")
                            nc.vector.match_replace(out=pdb[:, :], in_to_replace=v0[:, :],
                                                    in_values=pda[:, :], imm_value=NEG)
                            nc.vector.max(out=v1[:, :], in_=pdb[:, :])
                            nc.vector.max_index(out=iv[:, 8:16], in_max=v1[:, :], in_values=pdb[:, :])
                            nc.vector.match_replace(out=pda[:, :], in_to_replace=v1[:, :],
                                                    in_values=pdb[:, :], imm_value=NEG)
                            nc.vector.max(out=v2[:, :], in_=pda[:, :])
                            nc.vector.max_index(out=iv[:, 16:24], in_max=v2[:, :], in_values=pda[:, :])
                            # stage the 20 indices with one DMA (SP engine)
                            nc.sync.dma_start(stage[b, isl, 0:20], iv[:, 0:20])

                # ======== phase 2: gather + window-max + v + relu + pool ========
                for p in range(NPAIR):
                    if l < 3:
                        upair, vpair = uvt[p]
                        wrap = gpool.tile([128, N * K // 16], u16, tag="wrap")
                        for g in range(8):
                            bsrc = 2 * p + (0 if g < 4 else 1)
                            lin = stage[bsrc].rearrange("i t -> (i t)").rearrange(
                                "(c r) -> r c", r=16)
                            nc.sync.dma_start(wrap[g * 16:(g + 1) * 16, :], lin)
                        Mp = wkpool.tile([128, N], f32, tag="Mp")
                        NW = NI // K
                        for gc in range(NGC):
                            G = gpool.tile([128, NI], f32, tag="G", bufs=2)
                            if with_gather:
                                nc.gpsimd.ap_gather(
                                    out_ap=G[:, :], in_ap=upair[:, :],
                                    idxs_ap=wrap[:, gc * (NI // 16):(gc + 1) * (NI // 16)].bitcast(i16),
                                    channels=128, num_elems=N, d=1, num_idxs=NI)
                            else:
                                nc.vector.memset(G[:, :], 0.0)
                            window_max(G, Mp[:, gc * NW:(gc + 1) * NW])
                        nc.vector.tensor_add(Mp[:, :], Mp[:, :], vpair[:, :])
                        Fnext = fpool.tile([128, N], f32, tag=f"F{p}",
                                           name=f"F{p}_{l}", bufs=2)
                        nc.scalar.activation(Fnext[:, :], Mp[:, :], AF.Relu)
                        FBnext = fpool.tile([64, N], f32, tag=f"FB{p}",
                                            name=f"FB{p}_{l}", bufs=2)
                        nc.sync.dma_start(FBnext[:, :], Fnext[64:128, :])
                        # global max-pool for this layer
                        gp = selpool.tile([128, 1], f32, tag="gp")
                        nc.vector.reduce_max(out=gp[:, :], in_=Fnext[:, :], axis=AX.X)
                        nc.sync.dma_start(pooled_d[2 * p, l * 64:(l + 1) * 64], gp[0:64, :])
                        nc.sync.dma_start(pooled_d[2 * p + 1, l * 64:(l + 1) * 64], gp[64:128, :])
                        F[p], FB[p] = Fnext, FBnext
                    else:
                        # layer 4: gather the 64-ch pair-packed input features
                        # (cores 0-3 use unit A's indices, 4-7 unit B's), then
                        # rebuild u4 per unit with the zero-padded weights.
                        v4s = uvt[p]
                        wrap = gpool.tile([128, N * K // 16], u16, tag="wrap")
                        for g in range(8):
                            bsrc = 2 * p + (0 if g < 4 else 1)
                            lin = stage[bsrc].rearrange("i t -> (i t)").rearrange(
                                "(c r) -> r c", r=16)
                            nc.sync.dma_start(wrap[g * 16:(g + 1) * 16, :], lin)
                        Mps = [wkpool.tile([128, N], f32, tag="Mp", name=f"Mp4_{2*p+ui}")
                               for ui in range(2)]
                        NW = NI // K
                        for gc in range(NGC):
                            Gx = gpool.tile([128, NI], f32, tag="G", bufs=2)
                            if with_gather:
                                nc.gpsimd.ap_gather(
                                    out_ap=Gx[:, :], in_ap=F[p][:, :],
                                    idxs_ap=wrap[:, gc * (NI // 16):(gc + 1) * (NI // 16)].bitcast(i16),
                                    channels=128, num_elems=N, d=1, num_idxs=NI)
                            else:
                                nc.vector.memset(Gx[:, :], 0.0)
                            for ui in range(2):
                                wkey = 'wu3a' if ui == 0 else 'wu3b'
                                for hf in range(2):
                                    Gu = gpool.tile([128, NI // 2], f32, tag="Gu",
                                                    bufs=2)
                                    for sb in range(NI // 2 // 512):
                                        ssl = slice(hf * (NI // 2) + sb * 512,
                                                    hf * (NI // 2) + (sb + 1) * 512)
                                        ps = pspool.tile([128, 512], f32, tag="u4ps")
                                        nc.tensor.matmul(ps[:, :], wsb[wkey][:, :],
                                                         Gx[:, ssl], start=True, stop=True)
                                        nc.scalar.activation(
                                            Gu[:, sb * 512:(sb + 1) * 512], ps[:, :],
                                            AF.Copy)
                                        del ps
                                    window_max(Gu, Mps[ui][:, gc * NW + hf * (NW // 2):
                                                          gc * NW + (hf + 1) * (NW // 2)])
                        for ui in range(2):
                            b = 2 * p + ui
                            nc.vector.tensor_add(Mps[ui][:, :], Mps[ui][:, :], v4s[ui][:, :])
                            x4t = wkpool.tile([128, N], f32, tag="x4t", bufs=1)
                            nc.scalar.activation(x4t[:, :], Mps[ui][:, :], AF.Relu)
                            gp = selpool.tile([128, 1], f32, tag="gp")
                            nc.vector.reduce_max(out=gp[:, :], in_=x4t[:, :], axis=AX.X)
                            nc.sync.dma_start(pooled_d[b, 192:320], gp[:, :])

        # ================= MLP head (own pool scope) =================
        if not with_mlp:
            with tc.tile_pool(name="stub", bufs=1) as spool:
                so = spool.tile([40, BPC], f32, name="so")
                nc.sync.dma_start(so[:, :], pooled_d[:, 0:40].rearrange("b p -> p b"))
                nc.sync.dma_start(out_d, so[:, :])
        elif True:
          with (
            tc.tile_pool(name="mlp", bufs=1) as mpool,
            tc.tile_pool(name="mps", bufs=2, space="PSUM") as mpspool,
          ):
            pooledT = mpool.tile([128, 3, BPC], f32, tag="pooledT")
            for kc in range(3):
                kn = 128 if kc < 2 else 64
                nc.sync.dma_start(pooledT[0:kn, kc, :],
                                  pooled_d[:, kc * 128:kc * 128 + kn].rearrange("b p -> p b"))
            w1sb = mpool.tile([128, 3, 1024], f32, tag="w1sb")
            for kc in range(3):
                kn = 128 if kc < 2 else 64
                nc.sync.dma_start(w1sb[0:kn, kc, :], w1t[kc * 128:kc * 128 + kn, :])
            b1sb = mpool.tile([128, 8], f32, tag="b1sb")
            nc.sync.dma_start(b1sb[:, :], b1)
            h1 = mpool.tile([128, 8, BPC], f32, tag="h1")
            for mc in range(8):
                hp = mpspool.tile([128, BPC], f32, tag="acc")
                for kc in range(3):
                    kn = 128 if kc < 2 else 64
                    nc.tensor.matmul(hp[:, :], w1sb[0:kn, kc, mc * 128:(mc + 1) * 128],
                                     pooledT[0:kn, kc, :], start=(kc == 0), stop=(kc == 2))
                nc.scalar.activation(h1[:, mc, :], hp[:, :], AF.Relu,
                                     bias=b1sb[:, mc:mc + 1])
            w2sb = mpool.tile([128, 8, 512], f32, tag="w2sb")
            for kc in range(8):
                nc.sync.dma_start(w2sb[:, kc, :], w2t[kc * 128:(kc + 1) * 128, :])
            b2sb = mpool.tile([128, 4], f32, tag="b2sb")
            nc.sync.dma_start(b2sb[:, :], b2)
            h2 = mpool.tile([128, 4, BPC], f32, tag="h2")
            for mc in range(4):
                hp = mpspool.tile([128, BPC], f32, tag="acc")
                for kc in range(8):
                    nc.tensor.matmul(hp[:, :], w2sb[:, kc, mc * 128:(mc + 1) * 128],
                                     h1[:, kc, :], start=(kc == 0), stop=(kc == 7))
                nc.scalar.activation(h2[:, mc, :], hp[:, :], AF.Relu,
                                     bias=b2sb[:, mc:mc + 1])
            w3sb = mpool.tile([128, 4, 40], f32, tag="w3sb")
            for kc in range(4):
                nc.sync.dma_start(w3sb[:, kc, :], w3t[kc * 128:(kc + 1) * 128, :])
            b3sb = mpool.tile([40, 1], f32, tag="b3sb")
            nc.sync.dma_start(b3sb[:, :], b3)
            outp = mpspool.tile([40, BPC], f32, tag="acc")
            for kc in range(4):
                nc.tensor.matmul(outp[:, :], w3sb[:, kc, :], h2[:, kc, :],
                                 start=(kc == 0), stop=(kc == 3))
            outsb = mpool.tile([40, BPC], f32, tag="outsb")
            nc.scalar.activation(outsb[:, :], outp[:, :], AF.Identity, bias=b3sb[:, :])
            nc.sync.dma_start(out_d, outsb[:, :])

    nc.compile()
    return nc


# ---------------------------------------------------------------- exec bundle
def _get_bundle(**build_kwargs):
    """Compile the bass program and build the cached 8-core jitted callable."""
    key = ("bundle", tuple(sorted(build_kwargs.items())))
    if key in _CACHE:
        return _CACHE[key]

    import jax
    from jax.sharding import Mesh, PartitionSpec, NamedSharding
    from jax.experimental.shard_map import shard_map
    import concourse.bass2jax as b2j
    from concourse import mybir

    nc = _build_program(**build_kwargs)
    b2j.install_neuronx_cc_hook()

    partition_name = nc.partition_id_tensor.name if nc.partition_id_tensor else None
    in_names, out_names, out_avals, out_shapes = [], [], [], []
    for alloc in nc.m.functions[0].allocations:
        if not isinstance(alloc, mybir.MemoryLocationSet):
            continue
        name = alloc.memorylocations[0].name
        if alloc.kind == "ExternalInput":
            if name != partition_name:
                in_names.append(name)
        elif alloc.kind == "ExternalOutput":
            out_names.append(name)
            shape = tuple(alloc.tensor_shape)
            dtype = mybir.dt.np(alloc.dtype)
            out_avals.append(jax.core.ShapedArray(shape, dtype))
            out_shapes.append((shape, dtype))
    n_params = len(in_names)
    n_outs = len(out_avals)
    in_names_all = in_names + out_names
    if partition_name is not None:
        in_names_all.append(partition_name)
    donate = tuple(range(n_params, n_params + n_outs))

    def _body(*args):
        operands = list(args)
        if partition_name is not None:
            operands.append(b2j.partition_id_tensor())
        outs = b2j._bass_exec_p.bind(
            *operands,
            out_avals=tuple(out_avals),
            in_names=tuple(in_names_all),
            out_names=tuple(out_names),
            lowering_input_output_aliases=(),
            sim_require_finite=True,
            sim_require_nnan=True,
            nc=nc,
        )
        return tuple(outs)

    devices = jax.devices()[:NCORES]
    mesh = Mesh(np.asarray(devices), ("core",))
    sharding = NamedSharding(mesh, PartitionSpec("core"))
    sharded = jax.jit(
        shard_map(_body, mesh=mesh,
                  in_specs=(PartitionSpec("core"),) * (n_params + n_outs),
                  out_specs=(PartitionSpec("core"),) * n_outs,
                  check_rep=False),
        donate_argnums=donate, keep_unused=True)

    bundle = dict(nc=nc, sharded=sharded, in_names=in_names,
                  out_names=out_names, out_shapes=out_shapes,
                  sharding=sharding, n_params=n_params)
    _CACHE[key] = bundle
    return bundle


def _get_device_weights(bundle, inputs):
    """Device-resident concatenated weight buffers, verified per call."""
    import jax
    raw = {k: np.ascontiguousarray(np.asarray(inputs[k], np.float32))
           for k in _WNAMES}
    cached = _CACHE.get("wdev")
    if cached is not None:
        ok = all(np.array_equal(raw[k], cached["raw"][k]) for k in _WNAMES)
        if ok:
            return cached["dev"]
    w = _prep_weights(raw)
    dev = {}
    for name, arr in w.items():
        cat = np.concatenate([arr] * NCORES, axis=0)
        dev[name] = jax.device_put(cat, bundle["sharding"])
    jax.block_until_ready(list(dev.values()))
    _CACHE["wdev"] = dict(raw=raw, dev=dev)
    return dev


# ---------------------------------------------------------------- entry point
def _run(inputs, **spmd_kwargs):
    if spmd_kwargs:
        # debug/trace path: fall back to the uncached spmd runner
        return _run_uncached(inputs, **spmd_kwargs)
    import jax
    bundle = _get_bundle()
    dev_w = _get_device_weights(bundle, inputs)

    x = np.asarray(inputs['x'], dtype=np.float32)   # (32, 1024, 3)
    xT = np.ascontiguousarray(x.reshape(NCORES * BPC, N, 3).transpose(0, 2, 1))

    args = []
    for name in bundle["in_names"]:
        if name == "xT":
            args.append(xT)
        else:
            args.append(dev_w[name])
    for shape, dtype in bundle["out_shapes"]:
        args.append(np.zeros((NCORES * shape[0], *shape[1:]), dtype))

    outs = bundle["sharded"](*args)
    out0 = np.asarray(outs[bundle["out_names"].index("out")])  # (8*40, BPC)
    out = out0.reshape(NCORES, 40, BPC).transpose(0, 2, 1).reshape(B, 40)
    return out.astype(np.float32), _Result()


class _Result:
    exec_time_ns = None
    instructions_and_trace = None


def _run_uncached(inputs, **spmd_kwargs):
    key = "prog"
    if key not in _CACHE:
        _CACHE[key] = _build_program()
    nc = _CACHE[key]

    inputs = {k: np.asarray(v) for k, v in inputs.items()}
    w = _prep_weights(inputs)
    x = np.asarray(inputs['x'], dtype=np.float32)   # (32, 1024, 3)
    in_maps = []
    for c in range(NCORES):
        xs = x[c * BPC:(c + 1) * BPC]                       # (4, 1024, 3)
        m = {'xT': np.ascontiguousarray(xs.transpose(0, 2, 1)).astype(np.float32)}
        m.update({k: np.ascontiguousarray(v) for k, v in w.items()})
        in_maps.append(m)

    from concourse.bass_utils import run_bass_kernel_spmd
    res = run_bass_kernel_spmd(nc, in_maps, core_ids=list(range(NCORES)), **spmd_kwargs)
    out = np.concatenate([r['out'].T for r in res.results], axis=0)  # (32, 40)
    return out.astype(np.float32), res


def kernel(**inputs):
    return _run(inputs)[0]
